# revision 1
# baseline (speedup 1.0000x reference)
"""Multi-head causal self-attention on 8 TRN2 NeuronCores.

Problem (nn_MultiHeadAttention): B=2, T=2048, C=1024, H=16 heads, hs=64.
  q,k,v = per-head projections of x; causal softmax(q k^T / 8) v;
  concat heads; out = att @ Wo + bo.

Sharding: core c in 0..7 -> (batch b = c//4, head-group g = c%4, 4 heads each).
Each core computes Q/K/V + flash-style causal attention for its 4 heads on its
batch, normalized attention outputs are AllGathered across the 4 cores of the
same batch (replica groups [0-3], [4-7]), then each core computes a disjoint
256-column slice of the output projection (column-parallel Wo) + bias slice.
Host does a pure concat of the 8 disjoint output slices.

All matmuls run as float32r (single-pass fp32 PE mode, 4x faster than fp32).
Attention works in transposed layout throughout: Q^T/K^T [d, t], scores
S^T [s, t], P^T = exp(S^T/8) with causal mask, att^T [d, t] via
lhsT=[V_h | ones] (row 64 of the PSUM accumulator = softmax denominator).
Normalization multiplies by a PE-broadcast reciprocal row.

Scheduling notes: per-engine instruction order is static, so projection
(stage 1) and output-projection (stage 3) work is interleaved into the
attention head loops to fill PE bubbles left by the scores->exp->AV chain,
and stage-3 matmuls for t-block qb are emitted only during stage-2 of qb+1,
when their AllGathered inputs have already landed.
"""

import numpy as np
from contextlib import ExitStack

import concourse.bass as bass
import concourse.mybir as mybir
import concourse.tile as tile
from concourse import bacc
from concourse.bass_utils import run_bass_kernel_spmd

F32 = mybir.dt.float32
F32R = mybir.dt.float32r
EXP = mybir.ActivationFunctionType.Exp

N_CORES = 8
B = 2
T = 2048
C = 1024
NH = 16
HS = 64
E = 1024
GROUPS = 4          # head groups (tensor-parallel ranks per batch)
HPG = NH // GROUPS  # 4 heads per core
ES = E // GROUPS    # 256 output columns per core
HD = HPG * HS       # 256 local attention-output rows

P = 128             # partition tile
TBLK = 512          # t-block (matmul moving dim)
NTB = T // TBLK     # 4
NCT = C // P        # 8 contraction tiles for projections
NST = T // P        # 16 key tiles
VW = HS + 1         # V lhsT width per head (64 V cols + ones col)

REPLICA_GROUPS = [[0, 1, 2, 3], [4, 5, 6, 7]]


def build_nc(with_collective=True):
    """Build + compile the per-core SPMD program. Same program on all cores."""
    nc = bacc.Bacc(
        "TRN2", target_bir_lowering=False, debug=False, num_devices=N_CORES
    )

    xT = nc.dram_tensor("xT", [C, T], F32R, kind="ExternalInput").ap()
    wq = nc.dram_tensor("wq", [C, HD], F32R, kind="ExternalInput").ap()
    wk = nc.dram_tensor("wk", [C, HD], F32R, kind="ExternalInput").ap()
    wv = nc.dram_tensor("wv", [C, HD], F32R, kind="ExternalInput").ap()
    wo = nc.dram_tensor("wo", [E, ES], F32R, kind="ExternalInput").ap()
    bo = nc.dram_tensor("bo", [1, ES], F32R, kind="ExternalInput").ap()
    tri = nc.dram_tensor("tri", [P, P], F32R, kind="ExternalInput").ap()
    onesc = nc.dram_tensor("onesc", [1, P], F32R, kind="ExternalInput").ap()
    vones = nc.dram_tensor("vones", [P, HPG], F32R, kind="ExternalInput").ap()
    out = nc.dram_tensor("out", [T, ES], F32, kind="ExternalOutput").ap()

    with tile.TileContext(nc) as tc, ExitStack() as ctx:
        wp = ctx.enter_context(tc.tile_pool(name="wp", bufs=1))
        xp = ctx.enter_context(tc.tile_pool(name="xp", bufs=2))
        qkp = ctx.enter_context(tc.tile_pool(name="qkp", bufs=1))
        vp = ctx.enter_context(tc.tile_pool(name="vp", bufs=1))
        ptp = ctx.enter_context(tc.tile_pool(name="ptp", bufs=6))
        attp = ctx.enter_context(tc.tile_pool(name="attp", bufs=2))
        smp = ctx.enter_context(tc.tile_pool(name="smp", bufs=4))
        outp = ctx.enter_context(tc.tile_pool(name="outp", bufs=3))
        lhp = ctx.enter_context(tc.tile_pool(name="lhp", bufs=16))
        # PSUM: 8 banks total.  st2 [128,1024] = 2 banks x 2 bufs = 4,
        # attv 1 bank x 2, small (bc / out-proj) 1 bank x 2.
        ps2 = ctx.enter_context(tc.tile_pool(name="ps2", bufs=2, space="PSUM"))
        psB = ctx.enter_context(tc.tile_pool(name="psB", bufs=2, space="PSUM"))
        psC = ctx.enter_context(tc.tile_pool(name="psC", bufs=2, space="PSUM"))
        dramp = ctx.enter_context(tc.tile_pool(name="dramp", bufs=1, space="DRAM"))

        # ---- small constants ----
        ones = wp.tile([1, P], F32R, tag="ones")
        nc.sync.dma_start(ones[:], onesc[:])
        tri_sb = wp.tile([P, P], F32R, tag="tri")
        nc.sync.dma_start(tri_sb[:], tri[:])
        bias_sb = wp.tile([1, ES], F32R, tag="bias")

        w_sb = {n: [] for n in ("wq", "wk", "wv", "wo")}
        for name in ("wq", "wk", "wv", "wo"):
            for ci in range(NCT):
                w_sb[name].append(
                    wp.tile([P, ES], F32R, tag=f"{name}{ci}", name=f"{name}{ci}")
                )

        # x^T tiles per (c-tile, t-block), double-buffered across t-blocks:
        # x(tb) is only read by stage-1(tb), so two t-blocks' worth suffices
        xt_of = {}

        def alloc_xt(tb):
            xt_of[tb] = [
                xp.tile([P, TBLK], F32R, tag=f"x{ci}", name=f"x{ci}_{tb}")
                for ci in range(NCT)
            ]
            return xt_of[tb]

        # merged Q^T/K^T per head pair: col = tb*1024 + qk*512 + t_local
        # (pair p holds heads 2p (rows 0-63) and 2p+1 (rows 64-127))
        qkt = [qkp.tile([P, 2 * T], F32R, tag=f"qk{p_}", name=f"qk{p_}")
               for p_ in range(2)]

        def qt_slice(pr, r0, rn, t0, tn):
            tb, tl = t0 // TBLK, t0 % TBLK
            base = tb * 1024 + tl
            return qkt[pr][r0:r0 + rn, base:base + tn]

        def kt_slice(pr, r0, rn, s0, sn):
            tb, sl = s0 // TBLK, s0 % TBLK
            base = tb * 1024 + TBLK + sl
            return qkt[pr][r0:r0 + rn, base:base + sn]

        v_sb = [vp.tile([P, HPG * VW], F32R, tag=f"v{st}", name=f"v{st}")
                for st in range(NST)]

        # ---------------- stage-1 pieces ----------------
        def emit_x_dma(tb):
            ts_ = tb * TBLK
            xt = alloc_xt(tb)
            for ci in range(NCT):
                nc.sync.dma_start(
                    xt[ci][:], xT[ci * P:(ci + 1) * P, ts_:ts_ + TBLK])

        def emit_qk_proj(tb, pr, which):
            # one [128,512] accumulation on the psC "small" tag (see
            # emit_v_proj for why not st2); which=0 -> Q, which=1 -> K
            xt = xt_of[tb]
            wn = "wq" if which == 0 else "wk"
            ps = psC.tile([P, TBLK], F32, tag="small",
                          name=f"qkps{tb}_{pr}_{which}")
            for ci in range(NCT):
                nc.tensor.matmul(
                    ps[:],
                    lhsT=w_sb[wn][ci][:, pr * P:(pr + 1) * P],
                    rhs=xt[ci][:],
                    start=(ci == 0), stop=(ci == NCT - 1),
                )
            base = tb * 1024 + which * TBLK
            nc.vector.tensor_copy(qkt[pr][:, base:base + TBLK], ps[:])

        def emit_v_proj(st):
            # psC "small" tag, NOT ps2: a V filler holding an st2 slot for its
            # 8-matmul group would degrade the scores/exp pipeline to
            # single-buffering
            tb, sl = st // 4, (st % 4) * P
            xt = xt_of[tb]
            vps = psC.tile([P, TBLK], F32, tag="small", name=f"vps{st}")
            for ci in range(NCT):
                nc.tensor.matmul(
                    vps[:, 0:HD],
                    lhsT=xt[ci][:, sl:sl + P],
                    rhs=w_sb["wv"][ci][:],
                    start=(ci == 0), stop=(ci == NCT - 1),
                )
            nc.sync.dma_start(
                v_sb[st][:].rearrange("p (h x) -> p h x", h=HPG)[:, :, HS:VW],
                vones[:].rearrange("p (h o) -> p h o", o=1),
            )
            nc.vector.tensor_copy(
                v_sb[st][:].rearrange("p (h x) -> p h x", h=HPG)[:, :, 0:HS],
                vps[:, 0:HD].rearrange("p (h x) -> p h x", h=HPG),
            )

        def qk_chunks(tb):
            return [lambda tb=tb, pr=pr, w=w: emit_qk_proj(tb, pr, w)
                    for pr in range(2) for w in range(2)]

        def v_chunks(tb):
            return [lambda st=st: emit_v_proj(st)
                    for st in range(4 * tb, 4 * tb + 4)]

        # ------- stage-2 piece (one head PAIR of one t-block, jointly) ------
        def emit_headpair(qb, pr, attn_pair):
            """Process both heads of qkt pair `pr` together: the two score
            matmuls for one s-tile live in disjoint PE row-groups (lhsT rows
            0-63 vs 64-127) and run concurrently on hardware; one [128,1024]
            ACT exp covers both heads.  Yields once per s-tile so the driver
            can weave filler PE work into the exp-latency bubbles."""
            t0 = qb * TBLK
            ns = 4 * (qb + 1)
            attv = [
                psB.tile([VW, TBLK], F32, tag="attv", name=f"attv{qb}_{pr}_{par}")
                for par in range(2)
            ]
            for si in range(ns):
                diag = si * P >= t0
                ka = si * P - t0 if diag else 0
                stp = ps2.tile([P, 2 * TBLK], F32, tag="st2",
                               name=f"st{qb}_{pr}_{si}")
                for par in range(2):
                    r0 = par * HS
                    nc.tensor.matmul(
                        stp[:, par * TBLK:(par + 1) * TBLK],
                        lhsT=kt_slice(pr, r0, HS, si * P, P),
                        rhs=qt_slice(pr, r0, HS, t0, TBLK),
                        start=True, stop=True,
                    )
                pt = ptp.tile([P, 2 * TBLK], F32R, tag="pt",
                              name=f"pt{qb}_{pr}_{si}")
                if diag:
                    for par in range(2):
                        c0 = par * TBLK + ka
                        nc.scalar.activation(
                            pt[:, c0:(par + 1) * TBLK],
                            stp[:, c0:(par + 1) * TBLK], EXP, scale=0.125)
                        nc.vector.tensor_mul(
                            pt[:, c0:c0 + P], pt[:, c0:c0 + P], tri_sb[:])
                else:
                    nc.scalar.activation(pt[:], stp[:], EXP, scale=0.125)
                for par in range(2):
                    h = 2 * pr + par
                    nc.tensor.matmul(
                        attv[par][:, ka:TBLK],
                        lhsT=v_sb[si][:, h * VW:(h + 1) * VW],
                        rhs=pt[:, par * TBLK + ka:(par + 1) * TBLK],
                        start=(si == 0), stop=(si == ns - 1),
                    )
                yield
            # normalize: recip of denominator row, PE-broadcast, multiply
            for par in range(2):
                r0 = par * HS
                recip = smp.tile([1, TBLK], F32R, tag="recip")
                with nc.allow_low_precision(
                    reason="f32r reciprocal feeds PE broadcast; 19-bit "
                    "mantissa is ample for softmax denominators"
                ):
                    nc.vector.reciprocal(recip[:], attv[par][HS:HS + 1, :])
                bc = psC.tile([HS, TBLK], F32, tag="small",
                              name=f"bc{qb}_{pr}_{par}")
                nc.tensor.matmul(
                    bc[:], lhsT=ones[0:1, 0:HS], rhs=recip[:],
                    start=True, stop=True,
                )
                bcs = smp.tile([HS, TBLK], F32, tag="bcs")
                nc.vector.tensor_copy(bcs[:], bc[:])
                nc.vector.tensor_mul(
                    attn_pair[pr][r0:r0 + HS, :], attv[par][0:HS, :], bcs[:]
                )

        # ---------------- stage-3 piece (one t-tile of one t-block) ---------
        def emit_oproj_tt(qb, lh, tt):
            # lh[hdt] holds att^T rows for global heads (2*hdt, 2*hdt+1)...
            # here indexed so lh[hdt] pairs with w_sb["wo"][hdt]
            t0 = qb * TBLK
            op = psC.tile([P, ES], F32, tag="small", name=f"op{qb}_{tt}")
            nc.tensor.matmul(
                op[:], lhsT=ones[0:1, :], rhs=bias_sb[:],
                start=True, stop=False,
            )
            # pr0 tiles (even hdt) first: they arrive one AllGather earlier
            order = [0, 2, 4, 6, 1, 3, 5, 7]
            for i, hdt in enumerate(order):
                nc.tensor.matmul(
                    op[:],
                    lhsT=lh[hdt][:, tt * P:(tt + 1) * P],
                    rhs=w_sb["wo"][hdt][:],
                    start=False,
                    stop=(i == NCT - 1),
                )
            osb = outp.tile([P, ES], F32, tag="osb", name=f"osb{qb}_{tt}")
            # DVE, not ACT: out-proj fillers run inside exp-bound stretches
            nc.vector.tensor_copy(osb[:], op[:])
            nc.sync.dma_start(out[t0 + tt * P:t0 + (tt + 1) * P, :], osb[:])

        # --------- per-pair AllGather (pr = head pair 0/1 of this core) -----
        # Gathering one head-pair [128, 512] per collective: output rows are
        # rank-major, i.e. block g holds GLOBAL heads (4g+2pr, 4g+2pr+1) =
        # global hd-tile index 2g+pr.  lh list is indexed by wo-row tile.
        def emit_ag(qb, pr, attn_pair, lh):
            ag_in = dramp.tile([P, TBLK], F32R, tag=f"agin{qb}_{pr}")
            nc.sync.dma_start(ag_in[:], attn_pair[pr][:])
            ag_out = dramp.tile([GROUPS * P, TBLK], F32R, tag=f"agout{qb}_{pr}")
            if with_collective:
                nc.gpsimd.collective_compute(
                    "AllGather",
                    mybir.AluOpType.bypass,
                    replica_groups=REPLICA_GROUPS,
                    ins=[ag_in[:].opt()],
                    outs=[ag_out[:].opt()],
                )
            else:  # timing/sim variant: fake the AG with local DMA copies
                for g_ in range(GROUPS):
                    nc.sync.dma_start(
                        ag_out[g_ * P:(g_ + 1) * P, :], ag_in[:])
            for g_ in range(GROUPS):
                t_ = lhp.tile([P, TBLK], F32R, tag="lh",
                              name=f"lh{qb}_{pr}_{g_}")
                nc.sync.dma_start(t_[:], ag_out[g_ * P:(g_ + 1) * P, :])
                lh[2 * g_ + pr] = t_

        # ---------------- emission schedule ----------------
        # stage 1, t-block 0 (DMAs interleaved for fast start)
        xt0 = alloc_xt(0)
        for ci in range(NCT):
            nc.sync.dma_start(w_sb["wq"][ci][:], wq[ci * P:(ci + 1) * P, :])
            nc.sync.dma_start(xt0[ci][:], xT[ci * P:(ci + 1) * P, 0:TBLK])
        for ci in range(NCT):  # wk on HWDGE, wv on SWDGE: parallel sets
            nc.sync.dma_start(w_sb["wk"][ci][:], wk[ci * P:(ci + 1) * P, :])
        for ci in range(NCT):
            nc.sync.dma_start(w_sb["wv"][ci][:], wv[ci * P:(ci + 1) * P, :])
        for chunk in qk_chunks(0) + v_chunks(0):
            chunk()

        def drive_pair(qb, pr, attn_pair, vfill, fillers, stride, off=0):
            """Drive one head pair's s-loop, weaving V fillers (odd units,
            needed by this block's own diagonal s-tiles) and other fillers
            (every `stride` units starting after `off`)."""
            ctr = 0
            for _ in emit_headpair(qb, pr, attn_pair):
                ctr += 1
                if vfill and ctr % 2 == 1:
                    vfill.pop(0)()
                elif (fillers and ctr > off
                      and (ctr - off) % stride == 0):
                    fillers.pop(0)()

        lh_of = {}
        ap_of = {}

        def new_attn_pair(qb):
            ap_of[qb] = [
                attp.tile([P, TBLK], F32R, tag=f"attn{p_}", name=f"at{qb}_{p_}")
                for p_ in range(2)
            ]
            lh_of[qb] = [None] * NCT
            return ap_of[qb]

        def oproj_fillers(qb):
            return [(lambda tt=tt, q=qb: emit_oproj_tt(q, lh_of[q], tt))
                    for tt in range(4)]

        # ---- t-blocks 0 and 1: sequential ----
        for qb in (0, 1):
            emit_x_dma(qb + 1)
            if qb == 0:
                # wo/bias DMAs: needed only from stage 3 on, so they queue
                # behind the t-block-1 x loads
                for ci in range(NCT):
                    nc.sync.dma_start(
                        w_sb["wo"][ci][:], wo[ci * P:(ci + 1) * P, :])
                nc.sync.dma_start(bias_sb[:], bo[:])
            vfill = v_chunks(qb) if qb > 0 else []
            fillers = qk_chunks(qb + 1)
            if qb > 0:
                fillers += oproj_fillers(qb - 1)
            stride = max(2, (8 * (qb + 1)) // max(1, len(fillers)))
            ap = new_attn_pair(qb)
            drive_pair(qb, 0, ap, vfill, fillers, stride)
            emit_ag(qb, 0, ap, lh_of[qb])
            drive_pair(qb, 1, ap, vfill, fillers, stride)
            while vfill:
                vfill.pop(0)()
            while fillers:
                fillers.pop(0)()
            emit_ag(qb, 1, ap, lh_of[qb])

        # ---- t-blocks 2 and 3: interleaved at head-pair granularity ----
        # qb3's s-loops are exp(ACT)-bound while qb2 + the stage-1/3 fillers
        # are PE-rich; alternating their pairs averages the imbalance.
        emit_x_dma(3)
        ap2, ap3 = new_attn_pair(2), new_attn_pair(3)
        oq1 = oproj_fillers(1)
        # (2,0): V(tb2) on odd units; QK(tb3) + 2 oproj(qb1) strided
        drive_pair(2, 0, ap2, v_chunks(2), qk_chunks(3) + oq1[:2], 2)
        emit_ag(2, 0, ap2, lh_of[2])
        # (3,0): V(tb3) on odd units; rest of oproj(qb1) strided
        drive_pair(3, 0, ap3, v_chunks(3), oq1[2:], 6)
        emit_ag(3, 0, ap3, lh_of[3])
        # (2,1): nothing left to fill; exp backlog from (3,0) keeps ACT busy
        drive_pair(2, 1, ap2, [], [], 99)
        emit_ag(2, 1, ap2, lh_of[2])
        # (3,1): oproj(qb2) injected in the second half, once its
        # AllGathered inputs (issued just above) have landed
        oq2 = oproj_fillers(2)
        drive_pair(3, 1, ap3, [], oq2[:2], 3, off=8)
        emit_ag(3, 1, ap3, lh_of[3])
        while oq2:
            oq2.pop(0)()

        # tail: out-projection of the last t-block, two t-tiles per phase:
        # bias + pr0 hd-tiles (landed with the mid-block AllGather) first,
        # so PE has work while the final AllGather is in flight
        lhz = lh_of[NTB - 1]
        tz = (NTB - 1) * TBLK
        for grp in range(2):
            tts = (2 * grp, 2 * grp + 1)
            ops = {}
            for tt in tts:
                op = psC.tile([P, ES], F32, tag="small", name=f"opz{tt}")
                nc.tensor.matmul(
                    op[:], lhsT=ones[0:1, :], rhs=bias_sb[:],
                    start=True, stop=False,
                )
                for hdt in (0, 2, 4, 6):
                    nc.tensor.matmul(
                        op[:],
                        lhsT=lhz[hdt][:, tt * P:(tt + 1) * P],
                        rhs=w_sb["wo"][hdt][:],
                        start=False, stop=False,
                    )
                ops[tt] = op
            for tt in tts:
                for j, hdt in enumerate((1, 3, 5, 7)):
                    nc.tensor.matmul(
                        ops[tt][:],
                        lhsT=lhz[hdt][:, tt * P:(tt + 1) * P],
                        rhs=w_sb["wo"][hdt][:],
                        start=False, stop=(j == 3),
                    )
                osb = outp.tile([P, ES], F32, tag="osb", name=f"osbz{tt}")
                nc.vector.tensor_copy(osb[:], ops[tt][:])
                nc.sync.dma_start(
                    out[tz + tt * P:tz + (tt + 1) * P, :], osb[:])

    nc.compile()
    return nc


_NC_CACHE = {}


def _get_nc(with_collective=True):
    key = with_collective
    if key not in _NC_CACHE:
        _NC_CACHE[key] = build_nc(with_collective)
    return _NC_CACHE[key]


def make_in_maps(x, Wq, Wk, Wv, Wo, bo):
    tri = np.ascontiguousarray(np.triu(np.ones((P, P), dtype=np.float32)))
    onesc = np.ones((1, P), dtype=np.float32)
    vones = np.ones((P, HPG), dtype=np.float32)
    in_maps = []
    for c in range(N_CORES):
        b, g = c // GROUPS, c % GROUPS
        hs_ = slice(g * HPG, (g + 1) * HPG)
        in_maps.append({
            "xT": np.ascontiguousarray(x[b].T),
            "wq": np.ascontiguousarray(
                Wq[hs_].transpose(1, 0, 2).reshape(C, HD)),
            "wk": np.ascontiguousarray(
                Wk[hs_].transpose(1, 0, 2).reshape(C, HD)),
            "wv": np.ascontiguousarray(
                Wv[hs_].transpose(1, 0, 2).reshape(C, HD)),
            "wo": np.ascontiguousarray(Wo[:, g * ES:(g + 1) * ES]),
            "bo": np.ascontiguousarray(bo[g * ES:(g + 1) * ES].reshape(1, ES)),
            "tri": tri,
            "onesc": onesc,
            "vones": vones,
        })
    return in_maps


def kernel(x, Wq, Wk, Wv, Wo, bo):
    x = np.asarray(x, dtype=np.float32)
    Wq = np.asarray(Wq, dtype=np.float32)
    Wk = np.asarray(Wk, dtype=np.float32)
    Wv = np.asarray(Wv, dtype=np.float32)
    Wo = np.asarray(Wo, dtype=np.float32)
    bo = np.asarray(bo, dtype=np.float32)

    nc = _get_nc(with_collective=True)
    in_maps = make_in_maps(x, Wq, Wk, Wv, Wo, bo)
    res = run_bass_kernel_spmd(nc, in_maps, core_ids=list(range(N_CORES)))

    out = np.empty((B, T, E), dtype=np.float32)
    for c in range(N_CORES):
        b, g = c // GROUPS, c % GROUPS
        out[b, :, g * ES:(g + 1) * ES] = res.results[c]["out"]
    return out



# revision 40
# speedup vs baseline: 1.2481x; 1.2481x over previous
"""Multi-head causal self-attention on 8 TRN2 NeuronCores — fp8 DoubleRow.

Problem (nn_MultiHeadAttention): B=2, T=2048, C=1024, H=16 heads, hs=64.
  q,k,v = per-head projections of x; causal softmax(q k^T / 8) v;
  concat heads; out = att @ Wo + bo.

Sharding: core c in 0..7 -> (batch b = c//4, head-group g = c%4, 4 heads).
Per core: flash-style causal attention for its 4 heads, AllGather of the
normalized attention outputs across the 4 cores of the same batch, then a
disjoint 256-column slice of the output projection. Host concats slices.

Numerics (measured end-to-end rel-err 9.4e-3 vs 2e-2 budget):
  host:  x_hi=fp8(16x), x_lo=fp8(16x-x_hi); w{q,k,v}_hi=fp8(1024 W),
         w_lo=fp8(1024W - w_hi).  fp8 = e4m3; scales keep values in the
         e4m3 normal range (w~0.02 would otherwise land subnormal).
  QKV projections: fully error-compensated fp8 DoubleRow matmuls
         (w_hi.x_hi + w_hi.x_lo + w_lo.x_hi: 3 slot-products per K=128,
         12 DoubleRow instrs per 512-wide tile vs 8 f32r = 0.75x cycles,
         and each DoubleRow instr costs out_free/2 cycles = overall 2.67x
         fewer PE cycles than f32r).
  scores: q requantized to fp8 (q-side error only), k split hi/lo on
         device; DoubleRow lhsT=(k_hi|k_lo), rhs=(q|q dup) — 2x fewer
         cycles, diagonal tiles column-sliced to the causal region.
  P=exp(S/8): ACT, bf16 out, both head-pairs in one instruction.
  AV, output projection: bf16 (1:1 error transfer paths stay >=bf16).
  normalize: DVE reciprocal -> Pool partition_broadcast -> DVE multiply
         (no PE broadcast matmul, no PSUM->SBUF staging copy).

Scheduling: x/w are SBUF-resident (loaded once, ~46KB/partition), so
stage-1 (QKV) and stage-3 (out-proj) chunks are woven into the
scores->exp->AV s-loops to fill PE bubbles, as in the f32r baseline.
"""

import numpy as np
import ml_dtypes
from contextlib import ExitStack

import concourse.bass as bass
import concourse.mybir as mybir
import concourse.tile as tile
from concourse import bacc
from concourse.bass_utils import run_bass_kernel_spmd

F32 = mybir.dt.float32
F32R = mybir.dt.float32r
BF16 = mybir.dt.bfloat16
FP8 = mybir.dt.float8e4
EXP = mybir.ActivationFunctionType.Exp
DR = mybir.MatmulPerfMode.DoubleRow
E4 = ml_dtypes.float8_e4m3
BF = ml_dtypes.bfloat16

N_CORES = 8
B = 2
T = 2048
C = 1024
NH = 16
HS = 64
E = 1024
GROUPS = 4          # head groups (tensor-parallel ranks per batch)
HPG = NH // GROUPS  # 4 heads per core
ES = E // GROUPS    # 256 output columns per core
HD = HPG * HS       # 256 local attention-output rows

P = 128             # partition tile
TBLK = 512          # t-block (matmul moving dim)
NTB = T // TBLK     # 4
NCT = C // P        # 8 contraction tiles for projections
NST = T // P        # 16 key tiles
VW = HS + 1         # V lhsT width per head (64 V cols + ones col)

SX = 16.0           # x fp8 scale
SW = 1024.0         # weight fp8 scale
QK_CAST = 1.0 / 1024.0      # psum (x*w = 2^14 q) -> fp8 storage at 16 q
EXP_SCALE = 0.125 / 256.0   # scores psum = 256 * S_raw
V_CAST = 1.0 / 16384.0      # v psum -> natural-scale bf16

REPLICA_GROUPS = [[0, 1, 2, 3], [4, 5, 6, 7]]


def build_nc(with_collective=True):
    """Build + compile the per-core SPMD program. Same program on all cores."""
    nc = bacc.Bacc(
        "TRN2", target_bir_lowering=False, debug=False, num_devices=N_CORES
    )

    # x8: rows c=(ci,p), cols (hl, tb, t) — hi/lo-major so DoubleRow slot
    # pairs (hi,lo) and (ci,ci+1) are both expressible as free-dim strides
    x8 = nc.dram_tensor("x8", [C, 2 * T], FP8, kind="ExternalInput").ap()
    # w hi duplicated per ci (DoubleRow slots need physical duplication)
    wqh = nc.dram_tensor("wqh", [C, 2 * HD], FP8, kind="ExternalInput").ap()
    wkh = nc.dram_tensor("wkh", [C, 2 * HD], FP8, kind="ExternalInput").ap()
    wvh = nc.dram_tensor("wvh", [C, 2 * HD], FP8, kind="ExternalInput").ap()
    wql = nc.dram_tensor("wql", [C, HD], FP8, kind="ExternalInput").ap()
    wkl = nc.dram_tensor("wkl", [C, HD], FP8, kind="ExternalInput").ap()
    wvl = nc.dram_tensor("wvl", [C, HD], FP8, kind="ExternalInput").ap()
    wo = nc.dram_tensor("wo", [E, ES], BF16, kind="ExternalInput").ap()
    bo = nc.dram_tensor("bo", [1, ES], BF16, kind="ExternalInput").ap()
    tri2 = nc.dram_tensor("tri2", [P, 2 * P], BF16, kind="ExternalInput").ap()
    out = nc.dram_tensor("out", [T, ES], F32, kind="ExternalOutput").ap()

    with tile.TileContext(nc) as tc, ExitStack() as ctx:
        wp = ctx.enter_context(tc.tile_pool(name="wp", bufs=1))
        qkp = ctx.enter_context(tc.tile_pool(name="qkp", bufs=1))
        vp = ctx.enter_context(tc.tile_pool(name="vp", bufs=1))
        ptp = ctx.enter_context(tc.tile_pool(name="ptp", bufs=6))
        attp = ctx.enter_context(tc.tile_pool(name="attp", bufs=4))
        smp = ctx.enter_context(tc.tile_pool(name="smp", bufs=4))
        outp = ctx.enter_context(tc.tile_pool(name="outp", bufs=3))
        lhp = ctx.enter_context(tc.tile_pool(name="lhp", bufs=16))
        # PSUM: 8 banks. st2 [128,1024] = 2 banks x 2 bufs = 4,
        # attv 1 bank x 2, small (qkv proj / out-proj) 1 bank x 2.
        ps2 = ctx.enter_context(tc.tile_pool(name="ps2", bufs=2, space="PSUM"))
        psB = ctx.enter_context(tc.tile_pool(name="psB", bufs=2, space="PSUM"))
        psC = ctx.enter_context(tc.tile_pool(name="psC", bufs=2, space="PSUM"))
        dramp = ctx.enter_context(tc.tile_pool(name="dramp", bufs=1,
                                               space="DRAM"))

        # ---- SBUF-resident inputs ----
        x_sb = wp.tile([P, 2 * NCT * NTB * TBLK], FP8, tag="x")

        def x_ap():  # [p, hl, ci, tb, t]
            return x_sb[:].rearrange(
                "p (hl ci tb t) -> p hl ci tb t", hl=2, ci=NCT, tb=NTB)

        wqh_sb = wp.tile([P, NCT * 2 * HD], FP8, tag="wqh")
        wkh_sb = wp.tile([P, NCT * 2 * HD], FP8, tag="wkh")
        wvh_sb = wp.tile([P, NCT * 2 * HD], FP8, tag="wvh")
        wql_sb = wp.tile([P, NCT * HD], FP8, tag="wql")
        wkl_sb = wp.tile([P, NCT * HD], FP8, tag="wkl")
        wvl_sb = wp.tile([P, NCT * HD], FP8, tag="wvl")
        wo_sb = wp.tile([P, NCT * ES], BF16, tag="wo")
        bias_sb = wp.tile([1, ES], BF16, tag="bias")
        tri_sb = wp.tile([P, 2 * P], BF16, tag="tri")
        ones = wp.tile([1, P], BF16, tag="ones")

        def whi_ap(t):  # [p, ci, pr, two, m] (m=128 = pair cols)
            return t[:].rearrange(
                "p (ci pr two m) -> p ci pr two m", ci=NCT, pr=2, two=2)

        def wlo_ap(t):  # [p, ci, pr, m]
            return t[:].rearrange("p (ci pr m) -> p ci pr m", ci=NCT, pr=2)

        def wvh_ap():  # [p, ci, two, n] (n=256)
            return wvh_sb[:].rearrange(
                "p (ci two n) -> p ci two n", ci=NCT, two=2)

        def wvl_ap():  # [p, ci, n]
            return wvl_sb[:].rearrange("p (ci n) -> p ci n", ci=NCT)

        def wo_ap():
            return wo_sb[:].rearrange("p (ci n) -> p ci n", ci=NCT)

        # q fp8, duplicated for DoubleRow rhs slots: [p(2 heads), tb, 2, t]
        q8 = [qkp.tile([P, NTB * 2 * TBLK], FP8, tag=f"q8_{pr}",
                       name=f"q8_{pr}") for pr in range(2)]
        # k hi|lo per s-tile: [p(2 heads), st, 2, s(128)]
        k8 = [qkp.tile([P, NST * 2 * P], FP8, tag=f"k8_{pr}",
                       name=f"k8_{pr}") for pr in range(2)]

        def q8_ap(pr):
            return q8[pr][:].rearrange(
                "p (tb two t) -> p tb two t", tb=NTB, two=2)

        def k8_ap(pr):
            return k8[pr][:].rearrange(
                "p (st two s) -> p st two s", st=NST, two=2)

        # v (+ ones col) bf16: [p(s), st, h, VW]
        v_sb = vp.tile([P, NST * HPG * VW], BF16, tag="v")

        def v_ap():
            return v_sb[:].rearrange(
                "p (st h w) -> p st h w", st=NST, h=HPG)

        # ---------------- stage-1 pieces ----------------
        def emit_qk_proj(tb, pr, which):
            """q^T or k^T for head pair pr of t-block tb: [128, 512] PSUM
            via 12 fully-compensated fp8 DoubleRow matmuls, then requantize
            to fp8 (q duplicated by a Pool copy; k split hi/lo)."""
            wh_sb, wl_sb = ((wqh_sb, wql_sb), (wkh_sb, wkl_sb))[which]
            wh, wl = whi_ap(wh_sb), wlo_ap(wl_sb)
            xa = x_ap()
            ps = psC.tile([P, TBLK], F32, tag="small",
                          name=f"qkps{tb}_{pr}_{which}")
            n_in = 3 * (NCT // 2)
            i = 0
            for cp in range(NCT // 2):
                c0, c1 = 2 * cp, 2 * cp + 1
                for lhsT, rhs in (
                    (wh[:, c0, pr], xa[:, :, c0, tb]),          # w_hi.(x_hi+x_lo) c0
                    (wl[:, c0:c1 + 1, pr], xa[:, 0, c0:c1 + 1, tb]),  # w_lo.x_hi
                    (wh[:, c1, pr], xa[:, :, c1, tb]),          # w_hi.(x_hi+x_lo) c1
                ):
                    nc.tensor.matmul(
                        ps[:], lhsT=lhsT, rhs=rhs,
                        start=(i == 0), stop=(i == n_in - 1), perf_mode=DR,
                    )
                    i += 1
            with nc.allow_low_precision(reason="fp8 requantization of q/k "
                                        "is the measured-error design"):
                if which == 0:
                    nc.vector.tensor_scalar_mul(
                        q8_ap(pr)[:, tb, 0], ps[:], QK_CAST)
                    nc.gpsimd.tensor_copy(
                        q8_ap(pr)[:, tb, 1], q8_ap(pr)[:, tb, 0])
                else:
                    ka = k8_ap(pr)[:, 4 * tb:4 * tb + 4]  # [p, 4, 2, 128]
                    psv = ps[:].rearrange("p (st s) -> p st s", st=4)
                    nc.vector.tensor_scalar_mul(ka[:, :, 0], psv, QK_CAST)
                    nc.vector.scalar_tensor_tensor(
                        ka[:, :, 1], psv, QK_CAST, ka[:, :, 0],
                        op0=mybir.AluOpType.mult,
                        op1=mybir.AluOpType.subtract,
                    )

        def emit_v_proj(st):
            """v^T for s-tile st: [128(t), 256] PSUM via 12 compensated
            DoubleRow matmuls, cast to natural-scale bf16 into v_sb."""
            tb, sl = st // 4, (st % 4) * P
            xa = x_ap()
            wh, wl = wvh_ap(), wvl_ap()
            vps = psC.tile([P, HD], F32, tag="small", name=f"vps{st}")
            n_in = 3 * (NCT // 2)
            i = 0
            for cp in range(NCT // 2):
                c0, c1 = 2 * cp, 2 * cp + 1
                for lhsT, rhs in (
                    (xa[:, :, c0, tb, sl:sl + P], wh[:, c0]),
                    (xa[:, 0, c0:c1 + 1, tb, sl:sl + P], wl[:, c0:c1 + 1]),
                    (xa[:, :, c1, tb, sl:sl + P], wh[:, c1]),
                ):
                    nc.tensor.matmul(
                        vps[:], lhsT=lhsT, rhs=rhs,
                        start=(i == 0), stop=(i == n_in - 1), perf_mode=DR,
                    )
                    i += 1
            with nc.allow_low_precision(reason="bf16 V is the measured-"
                                        "error design"):
                nc.vector.tensor_scalar_mul(
                    v_ap()[:, st, :, 0:HS],
                    vps[:].rearrange("p (h d) -> p h d", h=HPG), V_CAST)

        def qk_chunks(tb):
            return [lambda tb=tb, pr=pr, w=w: emit_qk_proj(tb, pr, w)
                    for pr in range(2) for w in range(2)]

        def v_chunks(tb):
            return [lambda st=st: emit_v_proj(st)
                    for st in range(4 * tb, 4 * tb + 4)]

        # ------- stage-2 piece (one head PAIR of one t-block) ------
        def emit_headpair(qb, pr, attn_pair):
            """Causal attention s-loop for both heads of pair pr. Scores are
            k-compensated fp8 DoubleRow; one bf16 exp covers both heads;
            diagonal tiles are column-sliced to the causal region. Yields
            once per s-tile so the driver can weave PE filler work in."""
            t0 = qb * TBLK
            ns = 4 * (qb + 1)
            attv = [
                psB.tile([VW, TBLK], F32, tag="attv",
                         name=f"attv{qb}_{pr}_{par}")
                for par in range(2)
            ]
            def emit_av(si):
                ka = si * P - t0 if si * P >= t0 else 0
                ptv = pts[si]
                for par in range(2):
                    h = 2 * pr + par
                    nc.tensor.matmul(
                        attv[par][:, ka:TBLK],
                        lhsT=v_ap()[:, si, h],
                        rhs=ptv[:, par, ka:],
                        start=(si == 0), stop=(si == ns - 1),
                    )

            pts = {}
            for si in range(ns):
                diag = si * P >= t0
                ka = si * P - t0 if diag else 0
                stp = ps2.tile([P, 2 * TBLK], F32, tag="st2",
                               name=f"st{qb}_{pr}_{si}")
                stv = stp[:].rearrange("p (par t) -> p par t", par=2)
                for par in range(2):
                    r0 = par * HS
                    nc.tensor.matmul(
                        stv[:, par, ka:],
                        lhsT=k8_ap(pr)[r0:r0 + HS, si],
                        rhs=q8_ap(pr)[r0:r0 + HS, qb, :, ka:],
                        start=True, stop=True, perf_mode=DR,
                    )
                pt = ptp.tile([P, 2 * TBLK], BF16, tag="pt",
                              name=f"pt{qb}_{pr}_{si}")
                ptv = pt[:].rearrange("p (par t) -> p par t", par=2)
                pts[si] = ptv
                nc.scalar.activation(
                    ptv[:, :, ka:], stv[:, :, ka:], EXP, scale=EXP_SCALE)
                if diag:
                    with nc.allow_low_precision(reason="bf16 causal mask "
                                                "multiply on bf16 P"):
                        nc.vector.tensor_mul(
                            ptv[:, :, ka:ka + P], ptv[:, :, ka:ka + P],
                            tri_sb[:].rearrange("p (two s) -> p two s",
                                                two=2))
                # software pipeline: AV runs one s-tile behind, and PE
                # filler work (injected at the yield) sits between the
                # scores matmul and the AV so it runs during the exp flight
                yield
                if si > 0:
                    emit_av(si - 1)
            emit_av(ns - 1)
            # normalize: reciprocal of the denominator row (row 64 of attv),
            # Pool-broadcast across partitions, multiply into bf16 att.
            for par in range(2):
                r0 = par * HS
                recip = smp.tile([1, TBLK], F32, tag="recip")
                with nc.allow_low_precision(
                    reason="f32 reciprocal of softmax denominators"
                ):
                    nc.vector.reciprocal(recip[:], attv[par][HS:HS + 1, :])
                bc = smp.tile([HS, TBLK], F32, tag="bcast")
                nc.gpsimd.partition_broadcast(bc[:], recip[:])
                with nc.allow_low_precision(reason="bf16 attention output "
                                            "is the measured-error design"):
                    nc.vector.tensor_mul(
                        attn_pair[r0:r0 + HS, :], attv[par][0:HS, :], bc[:])

        # ---------------- stage-3 piece (one t-tile of one t-block) ---------
        def lh_slice(lh, hdt, c0, c1):
            lht, base = lh[hdt]
            return lht[:, base + c0:base + c1]

        def emit_oproj_tt(qb, lh, tt):
            t0 = qb * TBLK
            op = psC.tile([P, ES], F32, tag="small", name=f"op{qb}_{tt}")
            nc.tensor.matmul(
                op[:], lhsT=ones[0:1, :], rhs=bias_sb[:],
                start=True, stop=False,
            )
            # pr0 tiles (even hdt) first: they arrive one AllGather earlier
            order = [0, 2, 4, 6, 1, 3, 5, 7]
            for i, hdt in enumerate(order):
                nc.tensor.matmul(
                    op[:],
                    lhsT=lh_slice(lh, hdt, tt * P, (tt + 1) * P),
                    rhs=wo_ap()[:, hdt],
                    start=False,
                    stop=(i == NCT - 1),
                )
            osb = outp.tile([P, ES], F32, tag="osb", name=f"osb{qb}_{tt}")
            nc.vector.tensor_copy(osb[:], op[:])
            # output writes ride the Pool SWDGE queue, off the shared HWDGE
            nc.gpsimd.dma_start(out[t0 + tt * P:t0 + (tt + 1) * P, :], osb[:])

        # --------- per-pair AllGather (pr = head pair 0/1 of this core) -----
        # Output rows are rank-major: block g holds GLOBAL heads
        # (4g+2pr, 4g+2pr+1) = wo-row tile index 2g+pr.
        def emit_ag(qb, pr, attn_pair, lh):
            ag_out = dramp.tile([GROUPS * P, TBLK], BF16,
                                tag=f"agout{qb}_{pr}")
            if with_collective:
                ag_in = dramp.tile([P, TBLK], BF16, tag=f"agin{qb}_{pr}")
                nc.sync.dma_start(ag_in[:], attn_pair[:])
                nc.gpsimd.collective_compute(
                    "AllGather",
                    mybir.AluOpType.bypass,
                    replica_groups=REPLICA_GROUPS,
                    ins=[ag_in[:].opt()],
                    outs=[ag_out[:].opt()],
                )
            else:
                # timing/sim variant: byte-equivalent local DMAs (the input
                # staging write plus one write per gathered block)
                for g_ in range(GROUPS):
                    nc.sync.dma_start(
                        ag_out[g_ * P:(g_ + 1) * P, :], attn_pair[:])
            # per-block lh loads: block g only waits its own gather write,
            # so out-proj matmuls start as blocks land instead of waiting
            # for one big load
            for g_ in range(GROUPS):
                lhg = lhp.tile([P, TBLK], BF16, tag="lh",
                               name=f"lh{qb}_{pr}_{g_}")
                nc.sync.dma_start(lhg[:], ag_out[g_ * P:(g_ + 1) * P, :])
                lh[2 * g_ + pr] = (lhg, 0)
            if not with_collective:
                ag_in = dramp.tile([P, TBLK], BF16, tag=f"agin{qb}_{pr}")
                nc.sync.dma_start(ag_in[:], attn_pair[:])

        # ---------------- emission schedule ----------------
        # Upfront loads: weights for QK first, then x t-block by t-block,
        # then V/out-proj weights; constants via memset (no DMA).
        nc.gpsimd.memset(ones[:], 1.0)
        nc.gpsimd.memset(v_ap()[:, :, :, HS:VW], 1.0)

        def x_dma(tb):
            nc.sync.dma_start(
                x_ap()[:, :, :, tb],
                x8[:].rearrange("(ci p) (hl tb t) -> p hl ci tb t",
                                p=P, hl=2, tb=NTB)[:, :, :, tb],
            )

        def w_dma(t_sb, d):
            nc.sync.dma_start(
                t_sb[:].rearrange("p (ci f) -> p ci f", ci=NCT),
                d[:].rearrange("(ci p) f -> p ci f", p=P),
            )

        # wq + the first x t-block first, so stage-1 starts ASAP; wv before
        # x1 so the V(tb0) chunks don't stall the early-loop PE
        w_dma(wqh_sb, wqh)
        x_dma(0)
        w_dma(wql_sb, wql)
        nc.sync.dma_start(tri_sb[:], tri2[:])
        w_dma(wkh_sb, wkh)
        w_dma(wkl_sb, wkl)
        x_dma(1)
        w_dma(wvh_sb, wvh)
        w_dma(wvl_sb, wvl)
        x_dma(2)
        x_dma(3)
        nc.sync.dma_start(
            wo_sb[:].rearrange("p (ci f) -> p ci f", ci=NCT),
            wo[:].rearrange("(ci p) f -> p ci f", p=P),
        )
        nc.sync.dma_start(bias_sb[:], bo[:])

        for chunk in qk_chunks(0) + v_chunks(0):
            chunk()

        def drive_pair(qb, pr, attn_pair, vfill, fillers, stride, off=0):
            """Drive one head pair's s-loop, weaving V fillers (odd units)
            and other fillers (every `stride` units after `off`)."""
            ctr = 0
            for _ in emit_headpair(qb, pr, attn_pair):
                ctr += 1
                if vfill and ctr % 2 == 1:
                    vfill.pop(0)()
                elif (fillers and ctr > off
                      and (ctr - off) % stride == 0):
                    fillers.pop(0)()

        lh_of = {}
        ap_of = {}

        def new_attn_pair(qb):
            ap_of[qb] = [
                attp.tile([P, TBLK], BF16, tag=f"attn{p_}",
                          name=f"at{qb}_{p_}")
                for p_ in range(2)
            ]
            lh_of[qb] = [None] * NCT
            return ap_of[qb]

        def oproj_fillers(qb):
            return [(lambda tt=tt, q=qb: emit_oproj_tt(q, lh_of[q], tt))
                    for tt in range(4)]

        # ---- t-blocks 0 and 1: sequential. Stage-1 chunks (QK/V of later
        # t-blocks) fill these early loops; all out-projections are deferred
        # to the late loops, which have no stage-1 work left.
        ap0, ap1 = new_attn_pair(0), new_attn_pair(1)
        f0 = qk_chunks(1) + v_chunks(1)
        drive_pair(0, 0, ap0[0], [], f0, 1)
        emit_ag(0, 0, ap0[0], lh_of[0])
        drive_pair(0, 1, ap0[1], [], f0, 1)
        while f0:
            f0.pop(0)()
        emit_ag(0, 1, ap0[1], lh_of[0])

        f1 = qk_chunks(2) + v_chunks(2) + qk_chunks(3) + v_chunks(3)
        drive_pair(1, 0, ap1[0], [], f1, 1)
        emit_ag(1, 0, ap1[0], lh_of[1])
        drive_pair(1, 1, ap1[1], [], f1, 1)
        while f1:
            f1.pop(0)()
        emit_ag(1, 1, ap1[1], lh_of[1])

        # ---- t-blocks 2 and 3: interleaved at head-pair granularity,
        # fed by the deferred out-projections.
        ap2, ap3 = new_attn_pair(2), new_attn_pair(3)
        drive_pair(2, 0, ap2[0], [], oproj_fillers(0), 3)
        emit_ag(2, 0, ap2[0], lh_of[2])
        drive_pair(3, 0, ap3[0], [], oproj_fillers(1), 4)
        emit_ag(3, 0, ap3[0], lh_of[3])
        drive_pair(2, 1, ap2[1], [], [], 99)
        emit_ag(2, 1, ap2[1], lh_of[2])
        drive_pair(3, 1, ap3[1], [], [], 99)
        emit_ag(3, 1, ap3[1], lh_of[3])
        # oproj(2) runs here, covering the final AllGather's flight time
        for f in oproj_fillers(2):
            f()

        # tail: out-projection of the last t-block; bias + pr0 hd-tiles
        # (landed with the mid-block AllGather) first so PE has work while
        # the final AllGather is in flight.
        lhz = lh_of[NTB - 1]
        tz = (NTB - 1) * TBLK
        for grp in range(2):
            tts = (2 * grp, 2 * grp + 1)
            ops = {}
            for tt in tts:
                op = psC.tile([P, ES], F32, tag="small", name=f"opz{tt}")
                nc.tensor.matmul(
                    op[:], lhsT=ones[0:1, :], rhs=bias_sb[:],
                    start=True, stop=False,
                )
                for hdt in (0, 2, 4, 6):
                    nc.tensor.matmul(
                        op[:],
                        lhsT=lh_slice(lhz, hdt, tt * P, (tt + 1) * P),
                        rhs=wo_ap()[:, hdt],
                        start=False, stop=False,
                    )
                ops[tt] = op
            for tt in tts:
                for j, hdt in enumerate((1, 3, 5, 7)):
                    nc.tensor.matmul(
                        ops[tt][:],
                        lhsT=lh_slice(lhz, hdt, tt * P, (tt + 1) * P),
                        rhs=wo_ap()[:, hdt],
                        start=False, stop=(j == 3),
                    )
                osb = outp.tile([P, ES], F32, tag="osb", name=f"osbz{tt}")
                # endgame: ACT/DVE and SP/Pool alternate so the four final
                # stores drain in parallel instead of serializing
                if tt % 2 == 0:
                    nc.scalar.activation(
                        osb[:], ops[tt][:],
                        mybir.ActivationFunctionType.Copy, scale=1.0)
                    nc.sync.dma_start(
                        out[tz + tt * P:tz + (tt + 1) * P, :], osb[:])
                else:
                    nc.vector.tensor_copy(osb[:], ops[tt][:])
                    nc.gpsimd.dma_start(
                        out[tz + tt * P:tz + (tt + 1) * P, :], osb[:])

    nc.compile()
    return nc


_NC_CACHE = {}


def _get_nc(with_collective=True):
    key = with_collective
    if key not in _NC_CACHE:
        _NC_CACHE[key] = build_nc(with_collective)
    return _NC_CACHE[key]


def _f8(a):
    return a.astype(E4)


def _split8(a, scale):
    hi = _f8(scale * a)
    lo = _f8(scale * a - hi.astype(np.float32))
    return hi, lo


def make_in_maps(x, Wq, Wk, Wv, Wo, bo):
    tri = np.triu(np.ones((P, P), dtype=np.float32))
    tri2 = np.concatenate([tri, tri], axis=1).astype(BF)
    in_maps = []
    for c in range(N_CORES):
        b, g = c // GROUPS, c % GROUPS
        hs_ = slice(g * HPG, (g + 1) * HPG)

        # x8: [C, hl(2), tb(4), t(512)] -> [C, 2T]
        xT = np.ascontiguousarray(x[b].T)            # [C, T]
        x_hi, x_lo = _split8(xT, SX)
        x8 = np.stack([x_hi, x_lo], axis=1)          # [C, 2, T]
        x8 = x8.reshape(C, 2, NTB, TBLK).reshape(C, 2 * T)

        def prep_w(W):
            # W[hs_] -> [C, HD] in (pr, par, hs) column order
            Wl = W[hs_].transpose(1, 0, 2).reshape(C, HD)
            hi, lo = _split8(Wl, SW)
            # hi duplicated per pr block: [C, pr, 2, 128]
            hid = hi.reshape(C, 2, P)
            hid = np.stack([hid, hid], axis=2).reshape(C, 2 * HD)
            return np.ascontiguousarray(hid), np.ascontiguousarray(lo)

        wqh_, wql_ = prep_w(Wq)
        wkh_, wkl_ = prep_w(Wk)
        # V: hi duplicated as one [C, 2, 256] block (no pr split)
        Wvl_ = Wv[hs_].transpose(1, 0, 2).reshape(C, HD)
        v_hi, v_lo = _split8(Wvl_, SW)
        wvh_ = np.ascontiguousarray(
            np.stack([v_hi, v_hi], axis=1).reshape(C, 2 * HD))

        in_maps.append({
            "x8": np.ascontiguousarray(x8),
            "wqh": wqh_, "wkh": wkh_, "wvh": wvh_,
            "wql": wql_, "wkl": wkl_,
            "wvl": np.ascontiguousarray(v_lo),
            "wo": np.ascontiguousarray(Wo[:, g * ES:(g + 1) * ES]).astype(BF),
            "bo": np.ascontiguousarray(
                bo[g * ES:(g + 1) * ES].reshape(1, ES)).astype(BF),
            "tri2": tri2,
        })
    return in_maps


def kernel(x, Wq, Wk, Wv, Wo, bo):
    x = np.asarray(x, dtype=np.float32)
    Wq = np.asarray(Wq, dtype=np.float32)
    Wk = np.asarray(Wk, dtype=np.float32)
    Wv = np.asarray(Wv, dtype=np.float32)
    Wo = np.asarray(Wo, dtype=np.float32)
    bo = np.asarray(bo, dtype=np.float32)

    nc = _get_nc(with_collective=True)
    in_maps = make_in_maps(x, Wq, Wk, Wv, Wo, bo)
    res = run_bass_kernel_spmd(nc, in_maps, core_ids=list(range(N_CORES)))

    out = np.empty((B, T, E), dtype=np.float32)
    for c in range(N_CORES):
        b, g = c // GROUPS, c % GROUPS
        out[b, :, g * ES:(g + 1) * ES] = res.results[c]["out"]
    return out


# revision 59
# speedup vs baseline: 1.3123x; 1.0515x over previous
"""Multi-head causal self-attention on 8 TRN2 NeuronCores — fp8 DoubleRow.

Problem (nn_MultiHeadAttention): B=2, T=2048, C=1024, H=16 heads, hs=64.
  q,k,v = per-head projections of x; causal softmax(q k^T / 8) v;
  concat heads; out = att @ Wo + bo.

Sharding: core c in 0..7 -> (batch b = c//4, head-group g = c%4, 4 heads).
Per core: flash-style causal attention for its 4 heads, AllGather of the
normalized attention outputs across the 4 cores of the same batch, then a
disjoint 256-column slice of the output projection. Host concats slices.

Numerics (measured end-to-end rel-err 9.4e-3 vs 2e-2 budget):
  host:  x_hi=fp8(16x), x_lo=fp8(16x-x_hi); w{q,k,v}_hi=fp8(1024 W),
         w_lo=fp8(1024W - w_hi).  fp8 = e4m3; scales keep values in the
         e4m3 normal range (w~0.02 would otherwise land subnormal).
  QKV projections: fully error-compensated fp8 DoubleRow matmuls
         (w_hi.x_hi + w_hi.x_lo + w_lo.x_hi: 3 slot-products per K=128,
         12 DoubleRow instrs per 512-wide tile vs 8 f32r = 0.75x cycles,
         and each DoubleRow instr costs out_free/2 cycles = overall 2.67x
         fewer PE cycles than f32r).
  scores: q requantized to fp8 (q-side error only), k split hi/lo on
         device; DoubleRow lhsT=(k_hi|k_lo), rhs=(q|q dup) — 2x fewer
         cycles, diagonal tiles column-sliced to the causal region.
  P=exp(S/8): ACT, bf16 out, both head-pairs in one instruction.
  AV, output projection: bf16 (1:1 error transfer paths stay >=bf16).
  normalize: DVE reciprocal -> Pool partition_broadcast -> DVE multiply
         (no PE broadcast matmul, no PSUM->SBUF staging copy).

Scheduling: x/w are SBUF-resident (loaded once, ~46KB/partition), so
stage-1 (QKV) and stage-3 (out-proj) chunks are woven into the
scores->exp->AV s-loops to fill PE bubbles, as in the f32r baseline.
"""

import numpy as np
import ml_dtypes
from contextlib import ExitStack

import concourse.bass as bass
import concourse.mybir as mybir
import concourse.tile as tile
from concourse import bacc
from concourse.bass_utils import run_bass_kernel_spmd

F32 = mybir.dt.float32
F32R = mybir.dt.float32r
BF16 = mybir.dt.bfloat16
FP8 = mybir.dt.float8e4
EXP = mybir.ActivationFunctionType.Exp
DR = mybir.MatmulPerfMode.DoubleRow
E4 = ml_dtypes.float8_e4m3
BF = ml_dtypes.bfloat16

N_CORES = 8
B = 2
T = 2048
C = 1024
NH = 16
HS = 64
E = 1024
GROUPS = 4          # head groups (tensor-parallel ranks per batch)
HPG = NH // GROUPS  # 4 heads per core
ES = E // GROUPS    # 256 output columns per core
HD = HPG * HS       # 256 local attention-output rows

P = 128             # partition tile
TBLK = 512          # t-block (matmul moving dim)
NTB = T // TBLK     # 4
NCT = C // P        # 8 contraction tiles for projections
NST = T // P        # 16 key tiles
VW = HS + 1         # V lhsT width per head (64 V cols + ones col)

SX = 16.0           # x fp8 scale
SW = 1024.0         # weight fp8 scale
QK_CAST = 1.0 / 1024.0      # psum (x*w = 2^14 q) -> fp8 storage at 16 q
EXP_SCALE = 0.125 / 256.0   # scores psum = 256 * S_raw
V_CAST = 1.0 / 16384.0      # v psum -> natural-scale bf16

REPLICA_GROUPS = [[0, 1, 2, 3], [4, 5, 6, 7]]


def build_nc(with_collective=True):
    """Build + compile the per-core SPMD program. Same program on all cores."""
    nc = bacc.Bacc(
        "TRN2", target_bir_lowering=False, debug=False, num_devices=N_CORES
    )

    # x8: rows c=(ci,p), cols (hl, tb, t) — hi/lo-major so DoubleRow slot
    # pairs (hi,lo) and (ci,ci+1) are both expressible as free-dim strides
    x8 = nc.dram_tensor("x8", [C, 2 * T], FP8, kind="ExternalInput").ap()
    # w hi duplicated per ci (DoubleRow slots need physical duplication)
    wqh = nc.dram_tensor("wqh", [C, 2 * HD], FP8, kind="ExternalInput").ap()
    wkh = nc.dram_tensor("wkh", [C, 2 * HD], FP8, kind="ExternalInput").ap()
    wvh = nc.dram_tensor("wvh", [C, 2 * HD], FP8, kind="ExternalInput").ap()
    wql = nc.dram_tensor("wql", [C, HD], FP8, kind="ExternalInput").ap()
    wkl = nc.dram_tensor("wkl", [C, HD], FP8, kind="ExternalInput").ap()
    wvl = nc.dram_tensor("wvl", [C, HD], FP8, kind="ExternalInput").ap()
    wo = nc.dram_tensor("wo", [E, ES], BF16, kind="ExternalInput").ap()
    bo = nc.dram_tensor("bo", [1, ES], BF16, kind="ExternalInput").ap()
    tri2 = nc.dram_tensor("tri2", [P, 2 * P], BF16, kind="ExternalInput").ap()
    out = nc.dram_tensor("out", [T, ES], F32, kind="ExternalOutput").ap()

    with tile.TileContext(nc) as tc, ExitStack() as ctx:
        wp = ctx.enter_context(tc.tile_pool(name="wp", bufs=1))
        qkp = ctx.enter_context(tc.tile_pool(name="qkp", bufs=1))
        vp = ctx.enter_context(tc.tile_pool(name="vp", bufs=1))
        ptp = ctx.enter_context(tc.tile_pool(name="ptp", bufs=10))
        attp = ctx.enter_context(tc.tile_pool(name="attp", bufs=4))
        smp = ctx.enter_context(tc.tile_pool(name="smp", bufs=4))
        outp = ctx.enter_context(tc.tile_pool(name="outp", bufs=3))
        lhp = ctx.enter_context(tc.tile_pool(name="lhp", bufs=16))
        # PSUM: 8 banks. st2 [128,1024] = 2 banks x 2 bufs = 4,
        # attv 1 bank x 2, small (qkv proj / out-proj) 1 bank x 2.
        ps2 = ctx.enter_context(tc.tile_pool(name="ps2", bufs=2, space="PSUM"))
        psB = ctx.enter_context(tc.tile_pool(name="psB", bufs=2, space="PSUM"))
        psC = ctx.enter_context(tc.tile_pool(name="psC", bufs=2, space="PSUM"))
        dramp = ctx.enter_context(tc.tile_pool(name="dramp", bufs=1,
                                               space="DRAM"))

        # ---- SBUF-resident inputs ----
        x_sb = wp.tile([P, 2 * NCT * NTB * TBLK], FP8, tag="x")

        def x_ap():  # [p, hl, ci, tb, t]
            return x_sb[:].rearrange(
                "p (hl ci tb t) -> p hl ci tb t", hl=2, ci=NCT, tb=NTB)

        wqh_sb = wp.tile([P, NCT * 2 * HD], FP8, tag="wqh")
        wkh_sb = wp.tile([P, NCT * 2 * HD], FP8, tag="wkh")
        wvh_sb = wp.tile([P, NCT * 2 * HD], FP8, tag="wvh")
        wql_sb = wp.tile([P, NCT * HD], FP8, tag="wql")
        wkl_sb = wp.tile([P, NCT * HD], FP8, tag="wkl")
        wvl_sb = wp.tile([P, NCT * HD], FP8, tag="wvl")
        wo_sb = wp.tile([P, NCT * ES], BF16, tag="wo")
        bias_sb = wp.tile([1, ES], BF16, tag="bias")
        tri_sb = wp.tile([P, 2 * P], BF16, tag="tri")
        ones = wp.tile([1, P], BF16, tag="ones")

        def whi_ap(t):  # [p, ci, pr, two, m] (m=128 = pair cols)
            return t[:].rearrange(
                "p (ci pr two m) -> p ci pr two m", ci=NCT, pr=2, two=2)

        def wlo_ap(t):  # [p, ci, pr, m]
            return t[:].rearrange("p (ci pr m) -> p ci pr m", ci=NCT, pr=2)

        def wvh_ap():  # [p, ci, two, n] (n=256)
            return wvh_sb[:].rearrange(
                "p (ci two n) -> p ci two n", ci=NCT, two=2)

        def wvl_ap():  # [p, ci, n]
            return wvl_sb[:].rearrange("p (ci n) -> p ci n", ci=NCT)

        def wo_ap():
            return wo_sb[:].rearrange("p (ci n) -> p ci n", ci=NCT)

        # q fp8, duplicated for DoubleRow rhs slots: [p(2 heads), tb, 2, t]
        q8 = [qkp.tile([P, NTB * 2 * TBLK], FP8, tag=f"q8_{pr}",
                       name=f"q8_{pr}") for pr in range(2)]
        # k hi|lo per s-tile: [p(2 heads), st, 2, s(128)]
        k8 = [qkp.tile([P, NST * 2 * P], FP8, tag=f"k8_{pr}",
                       name=f"k8_{pr}") for pr in range(2)]

        def q8_ap(pr):
            return q8[pr][:].rearrange(
                "p (tb two t) -> p tb two t", tb=NTB, two=2)

        def k8_ap(pr):
            return k8[pr][:].rearrange(
                "p (st two s) -> p st two s", st=NST, two=2)

        # v (+ ones col) bf16: [p(s), st, h, VW]
        v_sb = vp.tile([P, NST * HPG * VW], BF16, tag="v")

        def v_ap():
            return v_sb[:].rearrange(
                "p (st h w) -> p st h w", st=NST, h=HPG)

        # ---------------- stage-1 pieces ----------------
        def emit_qk_proj(tb, pr, which):
            """q^T or k^T for head pair pr of t-block tb: [128, 512] PSUM
            via 12 fully-compensated fp8 DoubleRow matmuls, then requantize
            to fp8 (q duplicated by a Pool copy; k split hi/lo)."""
            wh_sb, wl_sb = ((wqh_sb, wql_sb), (wkh_sb, wkl_sb))[which]
            wh, wl = whi_ap(wh_sb), wlo_ap(wl_sb)
            xa = x_ap()
            ps = psC.tile([P, TBLK], F32, tag="small",
                          name=f"qkps{tb}_{pr}_{which}")
            n_in = 3 * (NCT // 2)
            i = 0
            for cp in range(NCT // 2):
                c0, c1 = 2 * cp, 2 * cp + 1
                for lhsT, rhs in (
                    (wh[:, c0, pr], xa[:, :, c0, tb]),          # w_hi.(x_hi+x_lo) c0
                    (wl[:, c0:c1 + 1, pr], xa[:, 0, c0:c1 + 1, tb]),  # w_lo.x_hi
                    (wh[:, c1, pr], xa[:, :, c1, tb]),          # w_hi.(x_hi+x_lo) c1
                ):
                    nc.tensor.matmul(
                        ps[:], lhsT=lhsT, rhs=rhs,
                        start=(i == 0), stop=(i == n_in - 1), perf_mode=DR,
                    )
                    i += 1
            with nc.allow_low_precision(reason="fp8 requantization of q/k "
                                        "is the measured-error design"):
                if which == 0:
                    nc.vector.tensor_scalar_mul(
                        q8_ap(pr)[:, tb, 0], ps[:], QK_CAST)
                    nc.gpsimd.tensor_copy(
                        q8_ap(pr)[:, tb, 1], q8_ap(pr)[:, tb, 0])
                else:
                    ka = k8_ap(pr)[:, 4 * tb:4 * tb + 4]  # [p, 4, 2, 128]
                    psv = ps[:].rearrange("p (st s) -> p st s", st=4)
                    nc.vector.tensor_scalar_mul(ka[:, :, 0], psv, QK_CAST)
                    nc.vector.scalar_tensor_tensor(
                        ka[:, :, 1], psv, QK_CAST, ka[:, :, 0],
                        op0=mybir.AluOpType.mult,
                        op1=mybir.AluOpType.subtract,
                    )

        def emit_v_proj(st):
            """v^T for s-tile st: [128(t), 256] PSUM via 12 compensated
            DoubleRow matmuls, cast to natural-scale bf16 into v_sb."""
            tb, sl = st // 4, (st % 4) * P
            xa = x_ap()
            wh, wl = wvh_ap(), wvl_ap()
            vps = psC.tile([P, HD], F32, tag="small", name=f"vps{st}")
            n_in = 3 * (NCT // 2)
            i = 0
            for cp in range(NCT // 2):
                c0, c1 = 2 * cp, 2 * cp + 1
                for lhsT, rhs in (
                    (xa[:, :, c0, tb, sl:sl + P], wh[:, c0]),
                    (xa[:, 0, c0:c1 + 1, tb, sl:sl + P], wl[:, c0:c1 + 1]),
                    (xa[:, :, c1, tb, sl:sl + P], wh[:, c1]),
                ):
                    nc.tensor.matmul(
                        vps[:], lhsT=lhsT, rhs=rhs,
                        start=(i == 0), stop=(i == n_in - 1), perf_mode=DR,
                    )
                    i += 1
            with nc.allow_low_precision(reason="bf16 V is the measured-"
                                        "error design"):
                nc.vector.tensor_scalar_mul(
                    v_ap()[:, st, :, 0:HS],
                    vps[:].rearrange("p (h d) -> p h d", h=HPG), V_CAST)

        def qk_chunks(tb):
            return [lambda tb=tb, pr=pr, w=w: emit_qk_proj(tb, pr, w)
                    for pr in range(2) for w in range(2)]

        def v_chunks(tb):
            return [lambda st=st: emit_v_proj(st)
                    for st in range(4 * tb, 4 * tb + 4)]

        # ------- stage-2 piece (one head PAIR of one t-block) ------
        def emit_headpair(qb, pr, attn_pair, cb0=0, cb1=TBLK):
            """Causal attention s-loop for both heads of pair pr, covering
            query columns [cb0, cb1) of the t-block. Scores are
            k-compensated fp8 DoubleRow; one bf16 exp covers both heads;
            diagonal tiles are column-sliced to the causal region. Yields
            once per s-tile so the driver can weave PE filler work in."""
            t0 = qb * TBLK
            cw = cb1 - cb0
            ns = (t0 + cb1) // P
            attv = [
                psB.tile([VW, cw], F32, tag="attv",
                         name=f"attv{qb}_{pr}_{par}_{cb0}")
                for par in range(2)
            ]

            def kae(si):
                ka = si * P - t0 if si * P >= t0 else 0
                return max(ka, cb0)

            def emit_av(si):
                ka = kae(si)
                ptv = pts[si]
                for par in range(2):
                    h = 2 * pr + par
                    nc.tensor.matmul(
                        attv[par][:, ka - cb0:cw],
                        lhsT=v_ap()[:, si, h],
                        rhs=ptv[:, par, ka:cb1],
                        start=(si == 0), stop=(si == ns - 1),
                    )

            pts = {}
            for si in range(ns):
                ka = kae(si)
                stp = ps2.tile([P, 2 * TBLK], F32, tag="st2",
                               name=f"st{qb}_{pr}_{si}_{cb0}")
                stv = stp[:].rearrange("p (par t) -> p par t", par=2)
                for par in range(2):
                    r0 = par * HS
                    nc.tensor.matmul(
                        stv[:, par, ka:cb1],
                        lhsT=k8_ap(pr)[r0:r0 + HS, si],
                        rhs=q8_ap(pr)[r0:r0 + HS, qb, :, ka:cb1],
                        start=True, stop=True, perf_mode=DR,
                    )
                pt = ptp.tile([P, 2 * TBLK], BF16, tag="pt",
                              name=f"pt{qb}_{pr}_{si}_{cb0}")
                ptv = pt[:].rearrange("p (par t) -> p par t", par=2)
                pts[si] = ptv
                nc.scalar.activation(
                    ptv[:, :, ka:cb1], stv[:, :, ka:cb1], EXP,
                    scale=EXP_SCALE)
                kd = si * P - t0
                if cb0 <= kd < cb1:
                    with nc.allow_low_precision(reason="bf16 causal mask "
                                                "multiply on bf16 P"):
                        nc.vector.tensor_mul(
                            ptv[:, :, kd:kd + P], ptv[:, :, kd:kd + P],
                            tri_sb[:].rearrange("p (two s) -> p two s",
                                                two=2))
                # software pipeline: AV runs several s-tiles behind, and PE
                # filler work (injected at the yield) sits between the
                # scores matmul and the AV so it runs during the exp flight
                yield
                if si > 5:
                    emit_av(si - 6)
            for s_ in range(max(0, ns - 6), ns):
                emit_av(s_)
            # normalize: reciprocal of the denominator row (row 64 of attv),
            # Pool-broadcast across partitions, multiply into bf16 att.
            for par in range(2):
                r0 = par * HS
                recip = smp.tile([1, cw], F32, tag="recip")
                with nc.allow_low_precision(
                    reason="f32 reciprocal of softmax denominators"
                ):
                    nc.vector.reciprocal(recip[:], attv[par][HS:HS + 1, :])
                bc = smp.tile([HS, cw], F32, tag="bcast")
                nc.gpsimd.partition_broadcast(bc[:], recip[:])
                with nc.allow_low_precision(reason="bf16 attention output "
                                            "is the measured-error design"):
                    nc.vector.tensor_mul(
                        attn_pair[r0:r0 + HS, cb0:cb1],
                        attv[par][0:HS, :], bc[:])

        # ---------------- stage-3 piece (one t-tile of one t-block) ---------
        def lh_slice(lh, hdt, c0, c1):
            lht, base = lh[hdt]
            return lht[:, base + c0:base + c1]

        def emit_oproj_tt(qb, lh, tt):
            t0 = qb * TBLK
            op = psC.tile([P, ES], F32, tag="small", name=f"op{qb}_{tt}")
            nc.tensor.matmul(
                op[:], lhsT=ones[0:1, :], rhs=bias_sb[:],
                start=True, stop=False,
            )
            # pr0 tiles (even hdt) first: they arrive one AllGather earlier
            order = [0, 2, 4, 6, 1, 3, 5, 7]
            for i, hdt in enumerate(order):
                nc.tensor.matmul(
                    op[:],
                    lhsT=lh_slice(lh, hdt, tt * P, (tt + 1) * P),
                    rhs=wo_ap()[:, hdt],
                    start=False,
                    stop=(i == NCT - 1),
                )
            osb = outp.tile([P, ES], F32, tag="osb", name=f"osb{qb}_{tt}")
            if qb >= 2:
                # endgame out-projections: ACT is idle (exps done) and the
                # Pool FIFO must stay clear for the partition_broadcasts
                nc.scalar.activation(
                    osb[:], op[:],
                    mybir.ActivationFunctionType.Copy, scale=1.0)
                nc.scalar.dma_start(
                    out[t0 + tt * P:t0 + (tt + 1) * P, :], osb[:])
            else:
                nc.vector.tensor_copy(osb[:], op[:])
                nc.gpsimd.dma_start(
                    out[t0 + tt * P:t0 + (tt + 1) * P, :], osb[:])

        # --------- per-pair AllGather (pr = head pair 0/1 of this core) -----
        # Output rows are rank-major: block g holds GLOBAL heads
        # (4g+2pr, 4g+2pr+1) = wo-row tile index 2g+pr.
        def emit_ag(qb, pr, attn_pair, lh):
            ag_out = dramp.tile([GROUPS * P, TBLK], BF16,
                                tag=f"agout{qb}_{pr}")
            if with_collective:
                ag_in = dramp.tile([P, TBLK], BF16, tag=f"agin{qb}_{pr}")
                nc.sync.dma_start(ag_in[:], attn_pair[:])
                nc.gpsimd.collective_compute(
                    "AllGather",
                    mybir.AluOpType.bypass,
                    replica_groups=REPLICA_GROUPS,
                    ins=[ag_in[:].opt()],
                    outs=[ag_out[:].opt()],
                )
            else:
                # timing/sim variant: byte-equivalent local DMAs (the input
                # staging write plus one write per gathered block)
                for g_ in range(GROUPS):
                    nc.sync.dma_start(
                        ag_out[g_ * P:(g_ + 1) * P, :], attn_pair[:])
            # per-block lh loads: block g only waits its own gather write,
            # so out-proj matmuls start as blocks land instead of waiting
            # for one big load
            for g_ in range(GROUPS):
                lhg = lhp.tile([P, TBLK], BF16, tag="lh",
                               name=f"lh{qb}_{pr}_{g_}")
                nc.sync.dma_start(lhg[:], ag_out[g_ * P:(g_ + 1) * P, :])
                lh[2 * g_ + pr] = (lhg, 0)
            if not with_collective:
                ag_in = dramp.tile([P, TBLK], BF16, tag=f"agin{qb}_{pr}")
                nc.sync.dma_start(ag_in[:], attn_pair[:])

        # ---------------- emission schedule ----------------
        # Upfront loads: weights for QK first, then x t-block by t-block,
        # then V/out-proj weights; constants via memset (no DMA).
        nc.gpsimd.memset(ones[:], 1.0)
        nc.gpsimd.memset(v_ap()[:, :, :, HS:VW], 1.0)

        # PE warmup: dependency-free matmuls on memset data ramp the PE to
        # full clock while the x/weight DMAs are in flight, so the first
        # real projection runs at 2.4 GHz instead of the cold p-state.
        wu = smp.tile([P, TBLK], BF16, tag="bcast", name="warmup_in")
        nc.gpsimd.memset(wu[:], 0.0)
        wups = psC.tile([P, TBLK], F32, tag="small", name="warmup_ps")
        NWU = 8
        for i in range(NWU):
            nc.tensor.matmul(
                wups[:], lhsT=wu[:, 0:P], rhs=wu[:],
                start=(i == 0), stop=(i == NWU - 1),
            )
        wuo = smp.tile([1, TBLK], F32, tag="recip", name="warmup_out")
        nc.vector.tensor_copy(wuo[:], wups[0:1, :])

        def x_dma(tb):
            nc.sync.dma_start(
                x_ap()[:, :, :, tb],
                x8[:].rearrange("(ci p) (hl tb t) -> p hl ci tb t",
                                p=P, hl=2, tb=NTB)[:, :, :, tb],
            )

        def w_dma(t_sb, d):
            nc.sync.dma_start(
                t_sb[:].rearrange("p (ci f) -> p ci f", ci=NCT),
                d[:].rearrange("(ci p) f -> p ci f", p=P),
            )

        # wq + the first x t-block first, so stage-1 starts ASAP; wv before
        # x1 so the V(tb0) chunks don't stall the early-loop PE
        w_dma(wqh_sb, wqh)
        x_dma(0)
        w_dma(wql_sb, wql)
        nc.sync.dma_start(tri_sb[:], tri2[:])
        w_dma(wkh_sb, wkh)
        w_dma(wkl_sb, wkl)
        x_dma(1)
        w_dma(wvh_sb, wvh)
        w_dma(wvl_sb, wvl)
        x_dma(2)
        x_dma(3)
        nc.sync.dma_start(
            wo_sb[:].rearrange("p (ci f) -> p ci f", ci=NCT),
            wo[:].rearrange("(ci p) f -> p ci f", p=P),
        )
        nc.sync.dma_start(bias_sb[:], bo[:])

        for chunk in qk_chunks(0) + v_chunks(0):
            chunk()

        def drive_pair(qb, pr, attn_pair, vfill, fillers, stride, off=0):
            """Drive one head pair's s-loop, weaving V fillers (odd units)
            and other fillers (every `stride` units after `off`)."""
            ctr = 0
            for _ in emit_headpair(qb, pr, attn_pair):
                ctr += 1
                if vfill and ctr % 2 == 1:
                    vfill.pop(0)()
                elif (fillers and ctr > off
                      and (ctr - off) % stride == 0):
                    fillers.pop(0)()

        lh_of = {}
        ap_of = {}

        def new_attn_pair(qb):
            ap_of[qb] = [
                attp.tile([P, TBLK], BF16, tag=f"attn{p_}",
                          name=f"at{qb}_{p_}")
                for p_ in range(2)
            ]
            lh_of[qb] = [None] * NCT
            return ap_of[qb]

        def oproj_fillers(qb):
            return [(lambda tt=tt, q=qb: emit_oproj_tt(q, lh_of[q], tt))
                    for tt in range(4)]

        # ---- t-blocks 0 and 1: sequential. Stage-1 chunks (QK/V of later
        # t-blocks) fill these early loops; all out-projections are deferred
        # to the late loops, which have no stage-1 work left.
        ap0, ap1 = new_attn_pair(0), new_attn_pair(1)
        f0 = qk_chunks(1) + v_chunks(1)
        drive_pair(0, 0, ap0[0], [], f0, 1)
        emit_ag(0, 0, ap0[0], lh_of[0])
        drive_pair(0, 1, ap0[1], [], f0, 1)
        while f0:
            f0.pop(0)()
        emit_ag(0, 1, ap0[1], lh_of[0])

        f1 = qk_chunks(2) + v_chunks(2)
        drive_pair(1, 0, ap1[0], [], f1, 1)
        emit_ag(1, 0, ap1[0], lh_of[1])
        drive_pair(1, 1, ap1[1], [], f1, 1)
        while f1:
            f1.pop(0)()
        emit_ag(1, 1, ap1[1], lh_of[1])

        # ---- t-blocks 2 and 3: interleaved at head-pair granularity.
        # Remaining stage-1 work and the deferred out-projections are
        # spread across these ACT-bound loops to keep the PE fed.
        ap2, ap3 = new_attn_pair(2), new_attn_pair(3)
        drive_pair(2, 0, ap2[0], [], qk_chunks(3) + v_chunks(3), 1)
        emit_ag(2, 0, ap2[0], lh_of[2])
        drive_pair(3, 0, ap3[0], [], oproj_fillers(0), 4)
        emit_ag(3, 0, ap3[0], lh_of[3])
        drive_pair(2, 1, ap2[1], [], oproj_fillers(1), 3)
        emit_ag(2, 1, ap2[1], lh_of[2])
        drive_pair(3, 1, ap3[1], [], oproj_fillers(2), 4)
        emit_ag(3, 1, ap3[1], lh_of[3])

        # tail: out-projection of the last t-block; bias + pr0 hd-tiles
        # (landed with the mid-block AllGather) first so PE has work while
        # the final AllGather is in flight.
        lhz = lh_of[NTB - 1]
        tz = (NTB - 1) * TBLK
        for grp in range(2):
            tts = (2 * grp, 2 * grp + 1)
            ops = {}
            for tt in tts:
                # tail groups live in the st2 banks: free after the last
                # exp, so their WAR guard resolves at exp time rather than
                # at the end of the final pair's normalize chain
                op = ps2.tile([P, ES], F32, tag="st2", name=f"opz{tt}")
                nc.tensor.matmul(
                    op[:], lhsT=ones[0:1, :], rhs=bias_sb[:],
                    start=True, stop=False,
                )
                for hdt in (0, 2, 4, 6):
                    nc.tensor.matmul(
                        op[:],
                        lhsT=lh_slice(lhz, hdt, tt * P, (tt + 1) * P),
                        rhs=wo_ap()[:, hdt],
                        start=False, stop=False,
                    )
                ops[tt] = op
            for tt in tts:
                for j, hdt in enumerate((1, 3, 5, 7)):
                    nc.tensor.matmul(
                        ops[tt][:],
                        lhsT=lh_slice(lhz, hdt, tt * P, (tt + 1) * P),
                        rhs=wo_ap()[:, hdt],
                        start=False, stop=(j == 3),
                    )
                osb = outp.tile([P, ES], F32, tag="osb", name=f"osbz{tt}")
                # endgame: ACT/DVE alternate so the final stores drain in
                # parallel instead of serializing on one engine
                if tt % 2 == 0:
                    nc.scalar.activation(
                        osb[:], ops[tt][:],
                        mybir.ActivationFunctionType.Copy, scale=1.0)
                    nc.scalar.dma_start(
                        out[tz + tt * P:tz + (tt + 1) * P, :], osb[:])
                else:
                    nc.vector.tensor_copy(osb[:], ops[tt][:])
                    nc.sync.dma_start(
                        out[tz + tt * P:tz + (tt + 1) * P, :], osb[:])

    nc.compile()
    return nc


_NC_CACHE = {}


def _get_nc(with_collective=True):
    key = with_collective
    if key not in _NC_CACHE:
        _NC_CACHE[key] = build_nc(with_collective)
    return _NC_CACHE[key]


def _f8(a):
    return a.astype(E4)


def _split8(a, scale):
    hi = _f8(scale * a)
    lo = _f8(scale * a - hi.astype(np.float32))
    return hi, lo


def make_in_maps(x, Wq, Wk, Wv, Wo, bo):
    tri = np.triu(np.ones((P, P), dtype=np.float32))
    tri2 = np.concatenate([tri, tri], axis=1).astype(BF)
    in_maps = []
    for c in range(N_CORES):
        b, g = c // GROUPS, c % GROUPS
        hs_ = slice(g * HPG, (g + 1) * HPG)

        # x8: [C, hl(2), tb(4), t(512)] -> [C, 2T]
        xT = np.ascontiguousarray(x[b].T)            # [C, T]
        x_hi, x_lo = _split8(xT, SX)
        x8 = np.stack([x_hi, x_lo], axis=1)          # [C, 2, T]
        x8 = x8.reshape(C, 2, NTB, TBLK).reshape(C, 2 * T)

        def prep_w(W):
            # W[hs_] -> [C, HD] in (pr, par, hs) column order
            Wl = W[hs_].transpose(1, 0, 2).reshape(C, HD)
            hi, lo = _split8(Wl, SW)
            # hi duplicated per pr block: [C, pr, 2, 128]
            hid = hi.reshape(C, 2, P)
            hid = np.stack([hid, hid], axis=2).reshape(C, 2 * HD)
            return np.ascontiguousarray(hid), np.ascontiguousarray(lo)

        wqh_, wql_ = prep_w(Wq)
        wkh_, wkl_ = prep_w(Wk)
        # V: hi duplicated as one [C, 2, 256] block (no pr split)
        Wvl_ = Wv[hs_].transpose(1, 0, 2).reshape(C, HD)
        v_hi, v_lo = _split8(Wvl_, SW)
        wvh_ = np.ascontiguousarray(
            np.stack([v_hi, v_hi], axis=1).reshape(C, 2 * HD))

        in_maps.append({
            "x8": np.ascontiguousarray(x8),
            "wqh": wqh_, "wkh": wkh_, "wvh": wvh_,
            "wql": wql_, "wkl": wkl_,
            "wvl": np.ascontiguousarray(v_lo),
            "wo": np.ascontiguousarray(Wo[:, g * ES:(g + 1) * ES]).astype(BF),
            "bo": np.ascontiguousarray(
                bo[g * ES:(g + 1) * ES].reshape(1, ES)).astype(BF),
            "tri2": tri2,
        })
    return in_maps


def kernel(x, Wq, Wk, Wv, Wo, bo):
    x = np.asarray(x, dtype=np.float32)
    Wq = np.asarray(Wq, dtype=np.float32)
    Wk = np.asarray(Wk, dtype=np.float32)
    Wv = np.asarray(Wv, dtype=np.float32)
    Wo = np.asarray(Wo, dtype=np.float32)
    bo = np.asarray(bo, dtype=np.float32)

    nc = _get_nc(with_collective=True)
    in_maps = make_in_maps(x, Wq, Wk, Wv, Wo, bo)
    res = run_bass_kernel_spmd(nc, in_maps, core_ids=list(range(N_CORES)))

    out = np.empty((B, T, E), dtype=np.float32)
    for c in range(N_CORES):
        b, g = c // GROUPS, c % GROUPS
        out[b, :, g * ES:(g + 1) * ES] = res.results[c]["out"]
    return out


# revision 78
# speedup vs baseline: 1.3151x; 1.0021x over previous
"""Multi-head causal self-attention on 8 TRN2 NeuronCores — fp8 DoubleRow.

Problem (nn_MultiHeadAttention): B=2, T=2048, C=1024, H=16 heads, hs=64.
  q,k,v = per-head projections of x; causal softmax(q k^T / 8) v;
  concat heads; out = att @ Wo + bo.

Sharding: core c in 0..7 -> (batch b = c//4, head-group g = c%4, 4 heads).
Per core: flash-style causal attention for its 4 heads, AllGather of the
normalized attention outputs across the 4 cores of the same batch, then a
disjoint 256-column slice of the output projection. Host concats slices.

Numerics (measured end-to-end rel-err 9.4e-3 vs 2e-2 budget):
  host:  x_hi=fp8(16x), x_lo=fp8(16x-x_hi); w{q,k,v}_hi=fp8(1024 W),
         w_lo=fp8(1024W - w_hi).  fp8 = e4m3; scales keep values in the
         e4m3 normal range (w~0.02 would otherwise land subnormal).
  QKV projections: fully error-compensated fp8 DoubleRow matmuls
         (w_hi.x_hi + w_hi.x_lo + w_lo.x_hi: 3 slot-products per K=128,
         12 DoubleRow instrs per 512-wide tile vs 8 f32r = 0.75x cycles,
         and each DoubleRow instr costs out_free/2 cycles = overall 2.67x
         fewer PE cycles than f32r).
  scores: q requantized to fp8 (q-side error only), k split hi/lo on
         device; DoubleRow lhsT=(k_hi|k_lo), rhs=(q|q dup) — 2x fewer
         cycles, diagonal tiles column-sliced to the causal region.
  P=exp(S/8): ACT, bf16 out, both head-pairs in one instruction.
  AV, output projection: bf16 (1:1 error transfer paths stay >=bf16).
  normalize: DVE reciprocal -> Pool partition_broadcast -> DVE multiply
         (no PE broadcast matmul, no PSUM->SBUF staging copy).

Scheduling: x/w are SBUF-resident (loaded once, ~46KB/partition), so
stage-1 (QKV) and stage-3 (out-proj) chunks are woven into the
scores->exp->AV s-loops to fill PE bubbles, as in the f32r baseline.
"""

import numpy as np
import ml_dtypes
from contextlib import ExitStack

import concourse.bass as bass
import concourse.mybir as mybir
import concourse.tile as tile
from concourse import bacc
from concourse.bass_utils import run_bass_kernel_spmd

F32 = mybir.dt.float32
F32R = mybir.dt.float32r
BF16 = mybir.dt.bfloat16
FP8 = mybir.dt.float8e4
EXP = mybir.ActivationFunctionType.Exp
DR = mybir.MatmulPerfMode.DoubleRow
E4 = ml_dtypes.float8_e4m3
BF = ml_dtypes.bfloat16

N_CORES = 8
B = 2
T = 2048
C = 1024
NH = 16
HS = 64
E = 1024
GROUPS = 4          # head groups (tensor-parallel ranks per batch)
HPG = NH // GROUPS  # 4 heads per core
ES = E // GROUPS    # 256 output columns per core
HD = HPG * HS       # 256 local attention-output rows

P = 128             # partition tile
TBLK = 512          # t-block (matmul moving dim)
NTB = T // TBLK     # 4
NCT = C // P        # 8 contraction tiles for projections
NST = T // P        # 16 key tiles
VW = HS + 1         # V lhsT width per head (64 V cols + ones col)

SX = 16.0           # x fp8 scale
SW = 1024.0         # weight fp8 scale
QK_CAST = 1.0 / 1024.0      # psum (x*w = 2^14 q) -> fp8 storage at 16 q
EXP_SCALE = 0.125 / 256.0   # scores psum = 256 * S_raw
V_CAST = 1.0 / 16384.0      # v psum -> natural-scale bf16

REPLICA_GROUPS = [[0, 1, 2, 3], [4, 5, 6, 7]]


def build_nc(with_collective=True):
    """Build + compile the per-core SPMD program. Same program on all cores."""
    nc = bacc.Bacc(
        "TRN2", target_bir_lowering=False, debug=False, num_devices=N_CORES
    )

    # x8: rows c=(ci,p), cols (hl, tb, t) — hi/lo-major so DoubleRow slot
    # pairs (hi,lo) and (ci,ci+1) are both expressible as free-dim strides
    x8 = nc.dram_tensor("x8", [C, 2 * T], FP8, kind="ExternalInput").ap()
    # w hi duplicated per ci (DoubleRow slots need physical duplication)
    wqh = nc.dram_tensor("wqh", [C, 2 * HD], FP8, kind="ExternalInput").ap()
    wkh = nc.dram_tensor("wkh", [C, 2 * HD], FP8, kind="ExternalInput").ap()
    wvh = nc.dram_tensor("wvh", [C, 2 * HD], FP8, kind="ExternalInput").ap()
    wql = nc.dram_tensor("wql", [C, HD], FP8, kind="ExternalInput").ap()
    wkl = nc.dram_tensor("wkl", [C, HD], FP8, kind="ExternalInput").ap()
    wvl = nc.dram_tensor("wvl", [C, HD], FP8, kind="ExternalInput").ap()
    wo = nc.dram_tensor("wo", [E, ES], BF16, kind="ExternalInput").ap()
    bo = nc.dram_tensor("bo", [1, ES], BF16, kind="ExternalInput").ap()
    tri2 = nc.dram_tensor("tri2", [P, 2 * P], BF16, kind="ExternalInput").ap()
    out = nc.dram_tensor("out", [T, ES], F32, kind="ExternalOutput").ap()

    with tile.TileContext(nc) as tc, ExitStack() as ctx:
        wp = ctx.enter_context(tc.tile_pool(name="wp", bufs=1))
        qkp = ctx.enter_context(tc.tile_pool(name="qkp", bufs=1))
        vp = ctx.enter_context(tc.tile_pool(name="vp", bufs=1))
        ptp = ctx.enter_context(tc.tile_pool(name="ptp", bufs=10))
        attp = ctx.enter_context(tc.tile_pool(name="attp", bufs=4))
        smp = ctx.enter_context(tc.tile_pool(name="smp", bufs=4))
        outp = ctx.enter_context(tc.tile_pool(name="outp", bufs=3))
        lhp = ctx.enter_context(tc.tile_pool(name="lhp", bufs=16))
        # PSUM: 8 banks. st2 [128,1024] = 2 banks x 2 bufs = 4,
        # attv 1 bank x 2, small (qkv proj / out-proj) 1 bank x 2.
        ps2 = ctx.enter_context(tc.tile_pool(name="ps2", bufs=2, space="PSUM"))
        psB = ctx.enter_context(tc.tile_pool(name="psB", bufs=2, space="PSUM"))
        psC = ctx.enter_context(tc.tile_pool(name="psC", bufs=2, space="PSUM"))
        dramp = ctx.enter_context(tc.tile_pool(name="dramp", bufs=1,
                                               space="DRAM"))

        # ---- SBUF-resident inputs ----
        x_sb = wp.tile([P, 2 * NCT * NTB * TBLK], FP8, tag="x")

        def x_ap():  # [p, hl, ci, tb, t]
            return x_sb[:].rearrange(
                "p (hl ci tb t) -> p hl ci tb t", hl=2, ci=NCT, tb=NTB)

        wqh_sb = wp.tile([P, NCT * 2 * HD], FP8, tag="wqh")
        wkh_sb = wp.tile([P, NCT * 2 * HD], FP8, tag="wkh")
        wvh_sb = wp.tile([P, NCT * 2 * HD], FP8, tag="wvh")
        wql_sb = wp.tile([P, NCT * HD], FP8, tag="wql")
        wkl_sb = wp.tile([P, NCT * HD], FP8, tag="wkl")
        wvl_sb = wp.tile([P, NCT * HD], FP8, tag="wvl")
        wo_sb = wp.tile([P, NCT * ES], BF16, tag="wo")
        bias_sb = wp.tile([1, ES], BF16, tag="bias")
        bias_bc = wp.tile([P, ES], BF16, tag="bias_bc")
        tri_sb = wp.tile([P, 2 * P], BF16, tag="tri")
        ones = wp.tile([1, P], BF16, tag="ones")

        def whi_ap(t):  # [p, ci, pr, two, m] (m=128 = pair cols)
            return t[:].rearrange(
                "p (ci pr two m) -> p ci pr two m", ci=NCT, pr=2, two=2)

        def wlo_ap(t):  # [p, ci, pr, m]
            return t[:].rearrange("p (ci pr m) -> p ci pr m", ci=NCT, pr=2)

        def wvh_ap():  # [p, ci, two, n] (n=256)
            return wvh_sb[:].rearrange(
                "p (ci two n) -> p ci two n", ci=NCT, two=2)

        def wvl_ap():  # [p, ci, n]
            return wvl_sb[:].rearrange("p (ci n) -> p ci n", ci=NCT)

        def wo_ap():
            return wo_sb[:].rearrange("p (ci n) -> p ci n", ci=NCT)

        # q fp8, duplicated for DoubleRow rhs slots: [p(2 heads), tb, 2, t]
        q8 = [qkp.tile([P, NTB * 2 * TBLK], FP8, tag=f"q8_{pr}",
                       name=f"q8_{pr}") for pr in range(2)]
        # k hi|lo per s-tile: [p(2 heads), st, 2, s(128)]
        k8 = [qkp.tile([P, NST * 2 * P], FP8, tag=f"k8_{pr}",
                       name=f"k8_{pr}") for pr in range(2)]

        def q8_ap(pr):
            return q8[pr][:].rearrange(
                "p (tb two t) -> p tb two t", tb=NTB, two=2)

        def k8_ap(pr):
            return k8[pr][:].rearrange(
                "p (st two s) -> p st two s", st=NST, two=2)

        # v (+ ones col) bf16: [p(s), st, h, VW]
        v_sb = vp.tile([P, NST * HPG * VW], BF16, tag="v")

        def v_ap():
            return v_sb[:].rearrange(
                "p (st h w) -> p st h w", st=NST, h=HPG)

        # ---------------- stage-1 pieces ----------------
        def emit_qk_proj(tb, pr, which):
            """q^T or k^T for head pair pr of t-block tb: [128, 512] PSUM
            via 12 fully-compensated fp8 DoubleRow matmuls, then requantize
            to fp8 (q duplicated by a Pool copy; k split hi/lo)."""
            wh_sb, wl_sb = ((wqh_sb, wql_sb), (wkh_sb, wkl_sb))[which]
            wh, wl = whi_ap(wh_sb), wlo_ap(wl_sb)
            xa = x_ap()
            ps = psC.tile([P, TBLK], F32, tag="small",
                          name=f"qkps{tb}_{pr}_{which}")
            n_in = 3 * (NCT // 2)
            i = 0
            for cp in range(NCT // 2):
                c0, c1 = 2 * cp, 2 * cp + 1
                for lhsT, rhs in (
                    (wh[:, c0, pr], xa[:, :, c0, tb]),          # w_hi.(x_hi+x_lo) c0
                    (wl[:, c0:c1 + 1, pr], xa[:, 0, c0:c1 + 1, tb]),  # w_lo.x_hi
                    (wh[:, c1, pr], xa[:, :, c1, tb]),          # w_hi.(x_hi+x_lo) c1
                ):
                    nc.tensor.matmul(
                        ps[:], lhsT=lhsT, rhs=rhs,
                        start=(i == 0), stop=(i == n_in - 1), perf_mode=DR,
                    )
                    i += 1
            with nc.allow_low_precision(reason="fp8 requantization of q/k "
                                        "is the measured-error design"):
                if which == 0:
                    nc.vector.tensor_scalar_mul(
                        q8_ap(pr)[:, tb, 0], ps[:], QK_CAST)
                    nc.gpsimd.tensor_copy(
                        q8_ap(pr)[:, tb, 1], q8_ap(pr)[:, tb, 0])
                else:
                    ka = k8_ap(pr)[:, 4 * tb:4 * tb + 4]  # [p, 4, 2, 128]
                    psv = ps[:].rearrange("p (st s) -> p st s", st=4)
                    nc.vector.tensor_scalar_mul(ka[:, :, 0], psv, QK_CAST)
                    nc.vector.scalar_tensor_tensor(
                        ka[:, :, 1], psv, QK_CAST, ka[:, :, 0],
                        op0=mybir.AluOpType.mult,
                        op1=mybir.AluOpType.subtract,
                    )

        def emit_v_proj(st):
            """v^T for s-tile st: [128(t), 256] PSUM via 12 compensated
            DoubleRow matmuls, cast to natural-scale bf16 into v_sb."""
            tb, sl = st // 4, (st % 4) * P
            xa = x_ap()
            wh, wl = wvh_ap(), wvl_ap()
            vps = psC.tile([P, HD], F32, tag="small", name=f"vps{st}")
            n_in = 3 * (NCT // 2)
            i = 0
            for cp in range(NCT // 2):
                c0, c1 = 2 * cp, 2 * cp + 1
                for lhsT, rhs in (
                    (xa[:, :, c0, tb, sl:sl + P], wh[:, c0]),
                    (xa[:, 0, c0:c1 + 1, tb, sl:sl + P], wl[:, c0:c1 + 1]),
                    (xa[:, :, c1, tb, sl:sl + P], wh[:, c1]),
                ):
                    nc.tensor.matmul(
                        vps[:], lhsT=lhsT, rhs=rhs,
                        start=(i == 0), stop=(i == n_in - 1), perf_mode=DR,
                    )
                    i += 1
            with nc.allow_low_precision(reason="bf16 V is the measured-"
                                        "error design"):
                nc.vector.tensor_scalar_mul(
                    v_ap()[:, st, :, 0:HS],
                    vps[:].rearrange("p (h d) -> p h d", h=HPG), V_CAST)

        def qk_chunks(tb):
            return [lambda tb=tb, pr=pr, w=w: emit_qk_proj(tb, pr, w)
                    for pr in range(2) for w in range(2)]

        def v_chunks(tb):
            return [lambda st=st: emit_v_proj(st)
                    for st in range(4 * tb, 4 * tb + 4)]

        # ------- stage-2 piece (one head PAIR of one t-block) ------
        def emit_headpair(qb, pr, attn_pair):
            """Causal attention s-loop for both heads of pair pr. Scores are
            k-compensated fp8 DoubleRow; one bf16 exp covers both heads;
            diagonal tiles are column-sliced to the causal region. Yields
            once per s-tile so the driver can weave PE filler work in."""
            t0 = qb * TBLK
            ns = 4 * (qb + 1)
            attv = [
                psB.tile([VW, TBLK], F32, tag="attv",
                         name=f"attv{qb}_{pr}_{par}")
                for par in range(2)
            ]
            def emit_av(si):
                ka = si * P - t0 if si * P >= t0 else 0
                ptv = pts[si]
                for par in range(2):
                    h = 2 * pr + par
                    nc.tensor.matmul(
                        attv[par][:, ka:TBLK],
                        lhsT=v_ap()[:, si, h],
                        rhs=ptv[:, par, ka:],
                        start=(si == 0), stop=(si == ns - 1),
                    )

            pts = {}
            for si in range(ns):
                diag = si * P >= t0
                ka = si * P - t0 if diag else 0
                stp = ps2.tile([P, 2 * TBLK], F32, tag="st2",
                               name=f"st{qb}_{pr}_{si}")
                stv = stp[:].rearrange("p (par t) -> p par t", par=2)
                for par in range(2):
                    r0 = par * HS
                    nc.tensor.matmul(
                        stv[:, par, ka:],
                        lhsT=k8_ap(pr)[r0:r0 + HS, si],
                        rhs=q8_ap(pr)[r0:r0 + HS, qb, :, ka:],
                        start=True, stop=True, perf_mode=DR,
                    )
                pt = ptp.tile([P, 2 * TBLK], BF16, tag="pt",
                              name=f"pt{qb}_{pr}_{si}")
                ptv = pt[:].rearrange("p (par t) -> p par t", par=2)
                pts[si] = ptv
                nc.scalar.activation(
                    ptv[:, :, ka:], stv[:, :, ka:], EXP, scale=EXP_SCALE)
                if diag:
                    with nc.allow_low_precision(reason="bf16 causal mask "
                                                "multiply on bf16 P"):
                        nc.vector.tensor_mul(
                            ptv[:, :, ka:ka + P], ptv[:, :, ka:ka + P],
                            tri_sb[:].rearrange("p (two s) -> p two s",
                                                two=2))
                # software pipeline: AV runs one s-tile behind, and PE
                # filler work (injected at the yield) sits between the
                # scores matmul and the AV so it runs during the exp flight
                yield
                if si > 5:
                    emit_av(si - 6)
            for s_ in range(max(0, ns - 6), ns):
                emit_av(s_)
            # normalize: reciprocal of the denominator row (row 64 of attv),
            # Pool-broadcast across partitions, multiply into bf16 att.
            for par in range(2):
                r0 = par * HS
                recip = smp.tile([1, TBLK], F32, tag="recip")
                with nc.allow_low_precision(
                    reason="f32 reciprocal of softmax denominators"
                ):
                    nc.vector.reciprocal(recip[:], attv[par][HS:HS + 1, :])
                bc = smp.tile([HS, TBLK], F32, tag="bcast")
                nc.gpsimd.partition_broadcast(bc[:], recip[:])
                with nc.allow_low_precision(reason="bf16 attention output "
                                            "is the measured-error design"):
                    nc.vector.tensor_mul(
                        attn_pair[r0:r0 + HS, :], attv[par][0:HS, :], bc[:])

        # ---------------- stage-3 piece (one t-tile of one t-block) ---------
        def lh_slice(lh, hdt, c0, c1):
            lht, base = lh[hdt]
            return lht[:, base + c0:base + c1]

        def emit_oproj_tt(qb, lh, tt):
            t0 = qb * TBLK
            op = psC.tile([P, ES], F32, tag="small", name=f"op{qb}_{tt}")
            # pr0 tiles (even hdt) first: they arrive one AllGather earlier
            order = [0, 2, 4, 6, 1, 3, 5, 7]
            for i, hdt in enumerate(order):
                nc.tensor.matmul(
                    op[:],
                    lhsT=lh_slice(lh, hdt, tt * P, (tt + 1) * P),
                    rhs=wo_ap()[:, hdt],
                    start=(i == 0),
                    stop=(i == NCT - 1),
                )
            osb = outp.tile([P, ES], F32, tag="osb", name=f"osb{qb}_{tt}")
            # bias is added during the PSUM->SBUF copy against a
            # pre-broadcast [128, ES] bias tile (no PE bias matmul)
            nc.vector.scalar_tensor_tensor(
                osb[:], op[:], 1.0, bias_bc[:],
                op0=mybir.AluOpType.mult, op1=mybir.AluOpType.add,
            )
            if qb >= 2:
                nc.scalar.dma_start(
                    out[t0 + tt * P:t0 + (tt + 1) * P, :], osb[:])
            else:
                nc.gpsimd.dma_start(
                    out[t0 + tt * P:t0 + (tt + 1) * P, :], osb[:])

        # --------- per-pair AllGather (pr = head pair 0/1 of this core) -----
        # Output rows are rank-major: block g holds GLOBAL heads
        # (4g+2pr, 4g+2pr+1) = wo-row tile index 2g+pr.
        def emit_ag(qb, pr, attn_pair, lh, last=False):
            # For the final AllGather the exps are done, so the otherwise
            # idle ACT DGE queue takes half the chain and the issue
            # overheads run in parallel with the SP queue's.
            q2 = nc.scalar if last else nc.sync
            ag_out = dramp.tile([GROUPS * P, TBLK], BF16,
                                tag=f"agout{qb}_{pr}")
            if with_collective:
                ag_in = dramp.tile([P, TBLK], BF16, tag=f"agin{qb}_{pr}")
                nc.sync.dma_start(ag_in[:], attn_pair[:])
                nc.gpsimd.collective_compute(
                    "AllGather",
                    mybir.AluOpType.bypass,
                    replica_groups=REPLICA_GROUPS,
                    ins=[ag_in[:].opt()],
                    outs=[ag_out[:].opt()],
                )
            else:
                # timing/sim variant: byte-equivalent local DMAs (the input
                # staging write plus one write per gathered block)
                for g_ in range(GROUPS):
                    eng = q2 if g_ % 2 else nc.sync
                    eng.dma_start(
                        ag_out[g_ * P:(g_ + 1) * P, :], attn_pair[:])
            # per-block lh loads: block g only waits its own gather write,
            # so out-proj matmuls start as blocks land instead of waiting
            # for one big load
            for g_ in range(GROUPS):
                lhg = lhp.tile([P, TBLK], BF16, tag="lh",
                               name=f"lh{qb}_{pr}_{g_}")
                q2.dma_start(lhg[:], ag_out[g_ * P:(g_ + 1) * P, :])
                lh[2 * g_ + pr] = (lhg, 0)
            if not with_collective:
                ag_in = dramp.tile([P, TBLK], BF16, tag=f"agin{qb}_{pr}")
                nc.sync.dma_start(ag_in[:], attn_pair[:])

        # ---------------- emission schedule ----------------
        # Upfront loads: weights for QK first, then x t-block by t-block,
        # then V/out-proj weights; constants via memset (no DMA).
        nc.gpsimd.memset(ones[:], 1.0)
        nc.gpsimd.memset(v_ap()[:, :, :, HS:VW], 1.0)

        # PE warmup: dependency-free matmuls on memset data ramp the PE to
        # full clock while the x/weight DMAs are in flight, so the first
        # real projection runs at 2.4 GHz instead of the cold p-state.
        wu = smp.tile([P, TBLK], BF16, tag="bcast", name="warmup_in")
        nc.gpsimd.memset(wu[:], 0.0)
        wups = psC.tile([P, TBLK], F32, tag="small", name="warmup_ps")
        NWU = 8
        for i in range(NWU):
            nc.tensor.matmul(
                wups[:], lhsT=wu[:, 0:P], rhs=wu[:],
                start=(i == 0), stop=(i == NWU - 1),
            )
        wuo = smp.tile([1, TBLK], F32, tag="recip", name="warmup_out")
        nc.vector.tensor_copy(wuo[:], wups[0:1, :])

        def x_dma(tb):
            nc.sync.dma_start(
                x_ap()[:, :, :, tb],
                x8[:].rearrange("(ci p) (hl tb t) -> p hl ci tb t",
                                p=P, hl=2, tb=NTB)[:, :, :, tb],
            )

        def w_dma(t_sb, d):
            nc.sync.dma_start(
                t_sb[:].rearrange("p (ci f) -> p ci f", ci=NCT),
                d[:].rearrange("(ci p) f -> p ci f", p=P),
            )

        # wq + the first x t-block first, so stage-1 starts ASAP; wv before
        # x1 so the V(tb0) chunks don't stall the early-loop PE
        w_dma(wqh_sb, wqh)
        x_dma(0)
        w_dma(wql_sb, wql)
        nc.sync.dma_start(tri_sb[:], tri2[:])
        w_dma(wkh_sb, wkh)
        w_dma(wkl_sb, wkl)
        x_dma(1)
        w_dma(wvh_sb, wvh)
        w_dma(wvl_sb, wvl)
        x_dma(2)
        x_dma(3)
        nc.sync.dma_start(
            wo_sb[:].rearrange("p (ci f) -> p ci f", ci=NCT),
            wo[:].rearrange("(ci p) f -> p ci f", p=P),
        )
        nc.sync.dma_start(bias_sb[:], bo[:])
        nc.gpsimd.partition_broadcast(bias_bc[:], bias_sb[:])

        for chunk in qk_chunks(0) + v_chunks(0):
            chunk()

        def drive_pair(qb, pr, attn_pair, vfill, fillers, stride, off=0):
            """Drive one head pair's s-loop, weaving V fillers (odd units)
            and other fillers (every `stride` units after `off`)."""
            ctr = 0
            for _ in emit_headpair(qb, pr, attn_pair):
                ctr += 1
                if vfill and ctr % 2 == 1:
                    vfill.pop(0)()
                elif (fillers and ctr > off
                      and (ctr - off) % stride == 0):
                    fillers.pop(0)()

        lh_of = {}
        ap_of = {}

        def new_attn_pair(qb):
            ap_of[qb] = [
                attp.tile([P, TBLK], BF16, tag=f"attn{p_}",
                          name=f"at{qb}_{p_}")
                for p_ in range(2)
            ]
            lh_of[qb] = [None] * NCT
            return ap_of[qb]

        def oproj_fillers(qb):
            return [(lambda tt=tt, q=qb: emit_oproj_tt(q, lh_of[q], tt))
                    for tt in range(4)]

        # ---- t-blocks 0 and 1: sequential. Stage-1 chunks (QK/V of later
        # t-blocks) fill these early loops; all out-projections are deferred
        # to the late loops, which have no stage-1 work left.
        ap0, ap1 = new_attn_pair(0), new_attn_pair(1)
        f0 = qk_chunks(1) + v_chunks(1)
        drive_pair(0, 0, ap0[0], [], f0, 1)
        emit_ag(0, 0, ap0[0], lh_of[0])
        drive_pair(0, 1, ap0[1], [], f0, 1)
        while f0:
            f0.pop(0)()
        emit_ag(0, 1, ap0[1], lh_of[0])

        f1 = qk_chunks(2) + v_chunks(2)
        drive_pair(1, 0, ap1[0], [], f1, 1)
        emit_ag(1, 0, ap1[0], lh_of[1])
        drive_pair(1, 1, ap1[1], [], f1, 1)
        while f1:
            f1.pop(0)()
        emit_ag(1, 1, ap1[1], lh_of[1])

        # ---- t-blocks 2 and 3: interleaved at head-pair granularity.
        # Remaining stage-1 work and the deferred out-projections are
        # spread across these ACT-bound loops to keep the PE fed.
        ap2, ap3 = new_attn_pair(2), new_attn_pair(3)
        drive_pair(2, 0, ap2[0], [], qk_chunks(3) + v_chunks(3), 1)
        emit_ag(2, 0, ap2[0], lh_of[2])
        drive_pair(3, 0, ap3[0], [], oproj_fillers(0), 4)
        emit_ag(3, 0, ap3[0], lh_of[3])
        drive_pair(2, 1, ap2[1], [], oproj_fillers(1), 3)
        emit_ag(2, 1, ap2[1], lh_of[2])
        drive_pair(3, 1, ap3[1], [], oproj_fillers(2), 4)

        # tail: out-projection of the last t-block; bias + pr0 hd-tiles
        # first so PE has work while the final AllGather is in flight.
        lhz = lh_of[NTB - 1]
        tz = (NTB - 1) * TBLK

        def open_tail_evens(tts):
            ops = {}
            for tt in tts:
                # tail groups live in the st2 banks: free after the last
                # exp, so their WAR guard resolves at exp time rather than
                # at the end of the final pair's normalize chain
                op = ps2.tile([P, ES], F32, tag="st2", name=f"opz{tt}")
                nc.tensor.matmul(
                    op[:], lhsT=ones[0:1, :], rhs=bias_sb[:],
                    start=True, stop=False,
                )
                for hdt in (0, 2, 4, 6):
                    nc.tensor.matmul(
                        op[:],
                        lhsT=lh_slice(lhz, hdt, tt * P, (tt + 1) * P),
                        rhs=wo_ap()[:, hdt],
                        start=False, stop=False,
                    )
                ops[tt] = op
            return ops

        # the first two t-tiles' even-half groups are emitted BEFORE the
        # final AllGather so their semaphore thresholds (and hence start
        # time) pre-date the gather chain they don't depend on
        ops = open_tail_evens((0, 1))
        emit_ag(3, 1, ap3[1], lh_of[3], last=True)
        for grp in range(2):
            tts = (2 * grp, 2 * grp + 1)
            if grp == 1:
                ops = open_tail_evens(tts)
            for tt in tts:
                for j, hdt in enumerate((1, 3, 5, 7)):
                    nc.tensor.matmul(
                        ops[tt][:],
                        lhsT=lh_slice(lhz, hdt, tt * P, (tt + 1) * P),
                        rhs=wo_ap()[:, hdt],
                        start=False, stop=(j == 3),
                    )
                osb = outp.tile([P, ES], F32, tag="osb", name=f"osbz{tt}")
                # endgame: ACT/DVE alternate so the final stores drain in
                # parallel instead of serializing on one engine
                if tt % 2 == 0:
                    nc.scalar.activation(
                        osb[:], ops[tt][:],
                        mybir.ActivationFunctionType.Copy, scale=1.0)
                    nc.scalar.dma_start(
                        out[tz + tt * P:tz + (tt + 1) * P, :], osb[:])
                else:
                    nc.vector.tensor_copy(osb[:], ops[tt][:])
                    nc.sync.dma_start(
                        out[tz + tt * P:tz + (tt + 1) * P, :], osb[:])

    nc.compile()
    return nc


_NC_CACHE = {}


def _get_nc(with_collective=True):
    key = with_collective
    if key not in _NC_CACHE:
        _NC_CACHE[key] = build_nc(with_collective)
    return _NC_CACHE[key]


def _f8(a):
    return a.astype(E4)


def _split8(a, scale):
    hi = _f8(scale * a)
    lo = _f8(scale * a - hi.astype(np.float32))
    return hi, lo


def make_in_maps(x, Wq, Wk, Wv, Wo, bo):
    tri = np.triu(np.ones((P, P), dtype=np.float32))
    tri2 = np.concatenate([tri, tri], axis=1).astype(BF)
    in_maps = []
    for c in range(N_CORES):
        b, g = c // GROUPS, c % GROUPS
        hs_ = slice(g * HPG, (g + 1) * HPG)

        # x8: [C, hl(2), tb(4), t(512)] -> [C, 2T]
        xT = np.ascontiguousarray(x[b].T)            # [C, T]
        x_hi, x_lo = _split8(xT, SX)
        x8 = np.stack([x_hi, x_lo], axis=1)          # [C, 2, T]
        x8 = x8.reshape(C, 2, NTB, TBLK).reshape(C, 2 * T)

        def prep_w(W):
            # W[hs_] -> [C, HD] in (pr, par, hs) column order
            Wl = W[hs_].transpose(1, 0, 2).reshape(C, HD)
            hi, lo = _split8(Wl, SW)
            # hi duplicated per pr block: [C, pr, 2, 128]
            hid = hi.reshape(C, 2, P)
            hid = np.stack([hid, hid], axis=2).reshape(C, 2 * HD)
            return np.ascontiguousarray(hid), np.ascontiguousarray(lo)

        wqh_, wql_ = prep_w(Wq)
        wkh_, wkl_ = prep_w(Wk)
        # V: hi duplicated as one [C, 2, 256] block (no pr split)
        Wvl_ = Wv[hs_].transpose(1, 0, 2).reshape(C, HD)
        v_hi, v_lo = _split8(Wvl_, SW)
        wvh_ = np.ascontiguousarray(
            np.stack([v_hi, v_hi], axis=1).reshape(C, 2 * HD))

        in_maps.append({
            "x8": np.ascontiguousarray(x8),
            "wqh": wqh_, "wkh": wkh_, "wvh": wvh_,
            "wql": wql_, "wkl": wkl_,
            "wvl": np.ascontiguousarray(v_lo),
            "wo": np.ascontiguousarray(Wo[:, g * ES:(g + 1) * ES]).astype(BF),
            "bo": np.ascontiguousarray(
                bo[g * ES:(g + 1) * ES].reshape(1, ES)).astype(BF),
            "tri2": tri2,
        })
    return in_maps


def kernel(x, Wq, Wk, Wv, Wo, bo):
    x = np.asarray(x, dtype=np.float32)
    Wq = np.asarray(Wq, dtype=np.float32)
    Wk = np.asarray(Wk, dtype=np.float32)
    Wv = np.asarray(Wv, dtype=np.float32)
    Wo = np.asarray(Wo, dtype=np.float32)
    bo = np.asarray(bo, dtype=np.float32)

    nc = _get_nc(with_collective=True)
    in_maps = make_in_maps(x, Wq, Wk, Wv, Wo, bo)
    res = run_bass_kernel_spmd(nc, in_maps, core_ids=list(range(N_CORES)))

    out = np.empty((B, T, E), dtype=np.float32)
    for c in range(N_CORES):
        b, g = c // GROUPS, c % GROUPS
        out[b, :, g * ES:(g + 1) * ES] = res.results[c]["out"]
    return out


# revision 85
# speedup vs baseline: 1.3253x; 1.0078x over previous
"""Multi-head causal self-attention on 8 TRN2 NeuronCores — fp8 DoubleRow.

Problem (nn_MultiHeadAttention): B=2, T=2048, C=1024, H=16 heads, hs=64.
  q,k,v = per-head projections of x; causal softmax(q k^T / 8) v;
  concat heads; out = att @ Wo + bo.

Sharding: core c in 0..7 -> (batch b = c//4, head-group g = c%4, 4 heads).
Per core: flash-style causal attention for its 4 heads, AllGather of the
normalized attention outputs across the 4 cores of the same batch, then a
disjoint 256-column slice of the output projection. Host concats slices.

Numerics (measured end-to-end rel-err 9.4e-3 vs 2e-2 budget):
  host:  x_hi=fp8(16x), x_lo=fp8(16x-x_hi); w{q,k,v}_hi=fp8(1024 W),
         w_lo=fp8(1024W - w_hi).  fp8 = e4m3; scales keep values in the
         e4m3 normal range (w~0.02 would otherwise land subnormal).
  QKV projections: fully error-compensated fp8 DoubleRow matmuls
         (w_hi.x_hi + w_hi.x_lo + w_lo.x_hi: 3 slot-products per K=128,
         12 DoubleRow instrs per 512-wide tile vs 8 f32r = 0.75x cycles,
         and each DoubleRow instr costs out_free/2 cycles = overall 2.67x
         fewer PE cycles than f32r).
  scores: q requantized to fp8 (q-side error only), k split hi/lo on
         device; DoubleRow lhsT=(k_hi|k_lo), rhs=(q|q dup) — 2x fewer
         cycles, diagonal tiles column-sliced to the causal region.
  P=exp(S/8): ACT, bf16 out, both head-pairs in one instruction.
  AV, output projection: bf16 (1:1 error transfer paths stay >=bf16).
  normalize: DVE reciprocal -> Pool partition_broadcast -> DVE multiply
         (no PE broadcast matmul, no PSUM->SBUF staging copy).

Scheduling: x/w are SBUF-resident (loaded once, ~46KB/partition), so
stage-1 (QKV) and stage-3 (out-proj) chunks are woven into the
scores->exp->AV s-loops to fill PE bubbles, as in the f32r baseline.
"""

import numpy as np
import ml_dtypes
from contextlib import ExitStack

import concourse.bass as bass
import concourse.mybir as mybir
import concourse.tile as tile
from concourse import bacc
from concourse.bass_utils import run_bass_kernel_spmd

F32 = mybir.dt.float32
F32R = mybir.dt.float32r
BF16 = mybir.dt.bfloat16
FP8 = mybir.dt.float8e4
EXP = mybir.ActivationFunctionType.Exp
DR = mybir.MatmulPerfMode.DoubleRow
E4 = ml_dtypes.float8_e4m3
BF = ml_dtypes.bfloat16

N_CORES = 8
B = 2
T = 2048
C = 1024
NH = 16
HS = 64
E = 1024
GROUPS = 4          # head groups (tensor-parallel ranks per batch)
HPG = NH // GROUPS  # 4 heads per core
ES = E // GROUPS    # 256 output columns per core
HD = HPG * HS       # 256 local attention-output rows

P = 128             # partition tile
TBLK = 512          # t-block (matmul moving dim)
NTB = T // TBLK     # 4
NCT = C // P        # 8 contraction tiles for projections
NST = T // P        # 16 key tiles
VW = HS + 1         # V lhsT width per head (64 V cols + ones col)

SX = 16.0           # x fp8 scale
SW = 1024.0         # weight fp8 scale
QK_CAST = 1.0 / 1024.0      # psum (x*w = 2^14 q) -> fp8 storage at 16 q
EXP_SCALE = 0.125 / 256.0   # scores psum = 256 * S_raw
V_CAST = 1.0 / 16384.0      # v psum -> natural-scale bf16

REPLICA_GROUPS = [[0, 1, 2, 3], [4, 5, 6, 7]]


def build_nc(with_collective=True):
    """Build + compile the per-core SPMD program. Same program on all cores."""
    nc = bacc.Bacc(
        "TRN2", target_bir_lowering=False, debug=False, num_devices=N_CORES
    )

    # x8: rows c=(ci,p), cols (hl, tb, t) — hi/lo-major so DoubleRow slot
    # pairs (hi,lo) and (ci,ci+1) are both expressible as free-dim strides
    x8 = nc.dram_tensor("x8", [C, 2 * T], FP8, kind="ExternalInput").ap()
    # w hi duplicated per ci (DoubleRow slots need physical duplication)
    wqh = nc.dram_tensor("wqh", [C, 2 * HD], FP8, kind="ExternalInput").ap()
    wkh = nc.dram_tensor("wkh", [C, 2 * HD], FP8, kind="ExternalInput").ap()
    wvh = nc.dram_tensor("wvh", [C, 2 * HD], FP8, kind="ExternalInput").ap()
    wql = nc.dram_tensor("wql", [C, HD], FP8, kind="ExternalInput").ap()
    wkl = nc.dram_tensor("wkl", [C, HD], FP8, kind="ExternalInput").ap()
    wvl = nc.dram_tensor("wvl", [C, HD], FP8, kind="ExternalInput").ap()
    wo = nc.dram_tensor("wo", [E, ES], BF16, kind="ExternalInput").ap()
    bo = nc.dram_tensor("bo", [1, ES], BF16, kind="ExternalInput").ap()
    tri2 = nc.dram_tensor("tri2", [P, 2 * P], BF16, kind="ExternalInput").ap()
    out = nc.dram_tensor("out", [T, ES], F32, kind="ExternalOutput").ap()

    with tile.TileContext(nc) as tc, ExitStack() as ctx:
        wp = ctx.enter_context(tc.tile_pool(name="wp", bufs=1))
        qkp = ctx.enter_context(tc.tile_pool(name="qkp", bufs=1))
        vp = ctx.enter_context(tc.tile_pool(name="vp", bufs=1))
        ptp = ctx.enter_context(tc.tile_pool(name="ptp", bufs=10))
        attp = ctx.enter_context(tc.tile_pool(name="attp", bufs=4))
        smp = ctx.enter_context(tc.tile_pool(name="smp", bufs=4))
        outp = ctx.enter_context(tc.tile_pool(name="outp", bufs=3))
        lhp = ctx.enter_context(tc.tile_pool(name="lhp", bufs=16))
        # PSUM: 8 banks. st2 [128,1024] = 2 banks x 2 bufs = 4,
        # attv 1 bank x 2, small (qkv proj / out-proj) 1 bank x 2.
        ps2 = ctx.enter_context(tc.tile_pool(name="ps2", bufs=2, space="PSUM"))
        psB = ctx.enter_context(tc.tile_pool(name="psB", bufs=2, space="PSUM"))
        psC = ctx.enter_context(tc.tile_pool(name="psC", bufs=2, space="PSUM"))
        dramp = ctx.enter_context(tc.tile_pool(name="dramp", bufs=1,
                                               space="DRAM"))

        # ---- SBUF-resident inputs ----
        x_sb = wp.tile([P, 2 * NCT * NTB * TBLK], FP8, tag="x")

        def x_ap():  # [p, hl, ci, tb, t]
            return x_sb[:].rearrange(
                "p (hl ci tb t) -> p hl ci tb t", hl=2, ci=NCT, tb=NTB)

        wqh_sb = wp.tile([P, NCT * 2 * HD], FP8, tag="wqh")
        wkh_sb = wp.tile([P, NCT * 2 * HD], FP8, tag="wkh")
        wvh_sb = wp.tile([P, NCT * 2 * HD], FP8, tag="wvh")
        wql_sb = wp.tile([P, NCT * HD], FP8, tag="wql")
        wkl_sb = wp.tile([P, NCT * HD], FP8, tag="wkl")
        wvl_sb = wp.tile([P, NCT * HD], FP8, tag="wvl")
        wo_sb = wp.tile([P, NCT * ES], BF16, tag="wo")
        bias_sb = wp.tile([1, ES], BF16, tag="bias")
        bias_bc = wp.tile([P, ES], BF16, tag="bias_bc")
        tri_sb = wp.tile([P, 2 * P], BF16, tag="tri")
        ones = wp.tile([1, P], BF16, tag="ones")

        def whi_ap(t):  # [p, ci, pr, two, m] (m=128 = pair cols)
            return t[:].rearrange(
                "p (ci pr two m) -> p ci pr two m", ci=NCT, pr=2, two=2)

        def wlo_ap(t):  # [p, ci, pr, m]
            return t[:].rearrange("p (ci pr m) -> p ci pr m", ci=NCT, pr=2)

        def wvh_ap():  # [p, ci, two, n] (n=256)
            return wvh_sb[:].rearrange(
                "p (ci two n) -> p ci two n", ci=NCT, two=2)

        def wvl_ap():  # [p, ci, n]
            return wvl_sb[:].rearrange("p (ci n) -> p ci n", ci=NCT)

        def wo_ap():
            return wo_sb[:].rearrange("p (ci n) -> p ci n", ci=NCT)

        # q fp8, duplicated for DoubleRow rhs slots: [p(2 heads), tb, 2, t]
        q8 = [qkp.tile([P, NTB * 2 * TBLK], FP8, tag=f"q8_{pr}",
                       name=f"q8_{pr}") for pr in range(2)]
        # k hi|lo per s-tile: [p(2 heads), st, 2, s(128)]
        k8 = [qkp.tile([P, NST * 2 * P], FP8, tag=f"k8_{pr}",
                       name=f"k8_{pr}") for pr in range(2)]

        def q8_ap(pr):
            return q8[pr][:].rearrange(
                "p (tb two t) -> p tb two t", tb=NTB, two=2)

        def k8_ap(pr):
            return k8[pr][:].rearrange(
                "p (st two s) -> p st two s", st=NST, two=2)

        # v (+ ones col) bf16: [p(s), st, h, VW]
        v_sb = vp.tile([P, NST * HPG * VW], BF16, tag="v")

        def v_ap():
            return v_sb[:].rearrange(
                "p (st h w) -> p st h w", st=NST, h=HPG)

        # ---------------- stage-1 pieces ----------------
        def emit_qk_proj(tb, pr, which):
            """q^T or k^T for head pair pr of t-block tb: [128, 512] PSUM
            via 12 fully-compensated fp8 DoubleRow matmuls, then requantize
            to fp8 (q duplicated by a Pool copy; k split hi/lo)."""
            wh_sb, wl_sb = ((wqh_sb, wql_sb), (wkh_sb, wkl_sb))[which]
            wh, wl = whi_ap(wh_sb), wlo_ap(wl_sb)
            xa = x_ap()
            ps = psC.tile([P, TBLK], F32, tag="small",
                          name=f"qkps{tb}_{pr}_{which}")
            n_in = 3 * (NCT // 2)
            i = 0
            for cp in range(NCT // 2):
                c0, c1 = 2 * cp, 2 * cp + 1
                for lhsT, rhs in (
                    (wh[:, c0, pr], xa[:, :, c0, tb]),          # w_hi.(x_hi+x_lo) c0
                    (wl[:, c0:c1 + 1, pr], xa[:, 0, c0:c1 + 1, tb]),  # w_lo.x_hi
                    (wh[:, c1, pr], xa[:, :, c1, tb]),          # w_hi.(x_hi+x_lo) c1
                ):
                    nc.tensor.matmul(
                        ps[:], lhsT=lhsT, rhs=rhs,
                        start=(i == 0), stop=(i == n_in - 1), perf_mode=DR,
                    )
                    i += 1
            with nc.allow_low_precision(reason="fp8 requantization of q/k "
                                        "is the measured-error design"):
                if which == 0:
                    nc.vector.tensor_scalar_mul(
                        q8_ap(pr)[:, tb, 0], ps[:], QK_CAST)
                    nc.gpsimd.tensor_copy(
                        q8_ap(pr)[:, tb, 1], q8_ap(pr)[:, tb, 0])
                else:
                    ka = k8_ap(pr)[:, 4 * tb:4 * tb + 4]  # [p, 4, 2, 128]
                    psv = ps[:].rearrange("p (st s) -> p st s", st=4)
                    nc.vector.tensor_scalar_mul(ka[:, :, 0], psv, QK_CAST)
                    nc.vector.scalar_tensor_tensor(
                        ka[:, :, 1], psv, QK_CAST, ka[:, :, 0],
                        op0=mybir.AluOpType.mult,
                        op1=mybir.AluOpType.subtract,
                    )

        def emit_v_proj(st):
            """v^T for s-tile st: [128(t), 256] PSUM via 12 compensated
            DoubleRow matmuls, cast to natural-scale bf16 into v_sb."""
            tb, sl = st // 4, (st % 4) * P
            xa = x_ap()
            wh, wl = wvh_ap(), wvl_ap()
            vps = psC.tile([P, HD], F32, tag="small", name=f"vps{st}")
            n_in = 3 * (NCT // 2)
            i = 0
            for cp in range(NCT // 2):
                c0, c1 = 2 * cp, 2 * cp + 1
                for lhsT, rhs in (
                    (xa[:, :, c0, tb, sl:sl + P], wh[:, c0]),
                    (xa[:, 0, c0:c1 + 1, tb, sl:sl + P], wl[:, c0:c1 + 1]),
                    (xa[:, :, c1, tb, sl:sl + P], wh[:, c1]),
                ):
                    nc.tensor.matmul(
                        vps[:], lhsT=lhsT, rhs=rhs,
                        start=(i == 0), stop=(i == n_in - 1), perf_mode=DR,
                    )
                    i += 1
            with nc.allow_low_precision(reason="bf16 V is the measured-"
                                        "error design"):
                nc.vector.tensor_scalar_mul(
                    v_ap()[:, st, :, 0:HS],
                    vps[:].rearrange("p (h d) -> p h d", h=HPG), V_CAST)

        def qk_chunks(tb):
            return [lambda tb=tb, pr=pr, w=w: emit_qk_proj(tb, pr, w)
                    for pr in range(2) for w in range(2)]

        def v_chunks(tb):
            return [lambda st=st: emit_v_proj(st)
                    for st in range(4 * tb, 4 * tb + 4)]

        # ------- stage-2 piece (one head PAIR of one t-block) ------
        def emit_headpair(qb, pr, attn_pair):
            """Causal attention s-loop for both heads of pair pr. Scores are
            k-compensated fp8 DoubleRow; one bf16 exp covers both heads;
            diagonal tiles are column-sliced to the causal region. Yields
            once per s-tile so the driver can weave PE filler work in."""
            t0 = qb * TBLK
            ns = 4 * (qb + 1)
            attv = [
                psB.tile([VW, TBLK], F32, tag="attv",
                         name=f"attv{qb}_{pr}_{par}")
                for par in range(2)
            ]
            def emit_av(si):
                ka = si * P - t0 if si * P >= t0 else 0
                ptv = pts[si]
                for par in range(2):
                    h = 2 * pr + par
                    nc.tensor.matmul(
                        attv[par][:, ka:TBLK],
                        lhsT=v_ap()[:, si, h],
                        rhs=ptv[:, par, ka:],
                        start=(si == 0), stop=(si == ns - 1),
                    )

            pts = {}
            for si in range(ns):
                diag = si * P >= t0
                ka = si * P - t0 if diag else 0
                stp = ps2.tile([P, 2 * TBLK], F32, tag="st2",
                               name=f"st{qb}_{pr}_{si}")
                stv = stp[:].rearrange("p (par t) -> p par t", par=2)
                for par in range(2):
                    r0 = par * HS
                    nc.tensor.matmul(
                        stv[:, par, ka:],
                        lhsT=k8_ap(pr)[r0:r0 + HS, si],
                        rhs=q8_ap(pr)[r0:r0 + HS, qb, :, ka:],
                        start=True, stop=True, perf_mode=DR,
                    )
                pt = ptp.tile([P, 2 * TBLK], BF16, tag="pt",
                              name=f"pt{qb}_{pr}_{si}")
                ptv = pt[:].rearrange("p (par t) -> p par t", par=2)
                pts[si] = ptv
                nc.scalar.activation(
                    ptv[:, :, ka:], stv[:, :, ka:], EXP, scale=EXP_SCALE)
                if diag:
                    with nc.allow_low_precision(reason="bf16 causal mask "
                                                "multiply on bf16 P"):
                        nc.vector.tensor_mul(
                            ptv[:, :, ka:ka + P], ptv[:, :, ka:ka + P],
                            tri_sb[:].rearrange("p (two s) -> p two s",
                                                two=2))
                # software pipeline: AV runs one s-tile behind, and PE
                # filler work (injected at the yield) sits between the
                # scores matmul and the AV so it runs during the exp flight
                yield
                if si > 5:
                    emit_av(si - 6)
            for s_ in range(max(0, ns - 6), ns):
                emit_av(s_)
            # normalize: reciprocal of the denominator row (row 64 of attv),
            # Pool-broadcast across partitions, multiply into bf16 att.
            for par in range(2):
                r0 = par * HS
                recip = smp.tile([1, TBLK], F32, tag="recip")
                with nc.allow_low_precision(
                    reason="f32 reciprocal of softmax denominators"
                ):
                    nc.vector.reciprocal(recip[:], attv[par][HS:HS + 1, :])
                bc = smp.tile([HS, TBLK], F32, tag="bcast")
                nc.gpsimd.partition_broadcast(bc[:], recip[:])
                with nc.allow_low_precision(reason="bf16 attention output "
                                            "is the measured-error design"):
                    nc.vector.tensor_mul(
                        attn_pair[r0:r0 + HS, :], attv[par][0:HS, :], bc[:])

        # ---------------- stage-3 piece (one t-tile of one t-block) ---------
        def lh_slice(lh, hdt, c0, c1):
            lht, base = lh[hdt]
            return lht[:, base + c0:base + c1]

        def emit_oproj_tt(qb, lh, tt):
            t0 = qb * TBLK
            op = psC.tile([P, ES], F32, tag="small", name=f"op{qb}_{tt}")
            # pr0 tiles (even hdt) first: they arrive one AllGather earlier
            order = [0, 2, 4, 6, 1, 3, 5, 7]
            for i, hdt in enumerate(order):
                nc.tensor.matmul(
                    op[:],
                    lhsT=lh_slice(lh, hdt, tt * P, (tt + 1) * P),
                    rhs=wo_ap()[:, hdt],
                    start=(i == 0),
                    stop=(i == NCT - 1),
                )
            osb = outp.tile([P, ES], F32, tag="osb", name=f"osb{qb}_{tt}")
            # bias is added during the PSUM->SBUF copy against a
            # pre-broadcast [128, ES] bias tile (no PE bias matmul)
            nc.vector.scalar_tensor_tensor(
                osb[:], op[:], 1.0, bias_bc[:],
                op0=mybir.AluOpType.mult, op1=mybir.AluOpType.add,
            )
            if qb >= 2:
                nc.scalar.dma_start(
                    out[t0 + tt * P:t0 + (tt + 1) * P, :], osb[:])
            else:
                nc.gpsimd.dma_start(
                    out[t0 + tt * P:t0 + (tt + 1) * P, :], osb[:])

        # --------- per-pair AllGather (pr = head pair 0/1 of this core) -----
        # Output rows are rank-major: block g holds GLOBAL heads
        # (4g+2pr, 4g+2pr+1) = wo-row tile index 2g+pr.
        def emit_ag(qb, pr, attn_pair, lh, last=False):
            # For the final AllGather the exps are done, so the otherwise
            # idle ACT DGE queue takes half the chain and the issue
            # overheads run in parallel with the SP queue's.
            q2 = nc.scalar if last else nc.sync
            ag_out = dramp.tile([GROUPS * P, TBLK], BF16,
                                tag=f"agout{qb}_{pr}")
            if with_collective:
                ag_in = dramp.tile([P, TBLK], BF16, tag=f"agin{qb}_{pr}")
                nc.sync.dma_start(ag_in[:], attn_pair[:])
                nc.gpsimd.collective_compute(
                    "AllGather",
                    mybir.AluOpType.bypass,
                    replica_groups=REPLICA_GROUPS,
                    ins=[ag_in[:].opt()],
                    outs=[ag_out[:].opt()],
                )
            else:
                # timing/sim variant: byte-equivalent local DMAs (the input
                # staging write plus one write per gathered block)
                for g_ in range(GROUPS):
                    eng = q2 if g_ % 2 else nc.sync
                    eng.dma_start(
                        ag_out[g_ * P:(g_ + 1) * P, :], attn_pair[:])
            # per-block lh loads: block g only waits its own gather write,
            # so out-proj matmuls start as blocks land instead of waiting
            # for one big load
            for g_ in range(GROUPS):
                lhg = lhp.tile([P, TBLK], BF16, tag="lh",
                               name=f"lh{qb}_{pr}_{g_}")
                q2.dma_start(lhg[:], ag_out[g_ * P:(g_ + 1) * P, :])
                lh[2 * g_ + pr] = (lhg, 0)
            if not with_collective:
                ag_in = dramp.tile([P, TBLK], BF16, tag=f"agin{qb}_{pr}")
                nc.sync.dma_start(ag_in[:], attn_pair[:])

        # ---------------- emission schedule ----------------
        # Upfront loads: weights for QK first, then x t-block by t-block,
        # then V/out-proj weights; constants via memset (no DMA).
        nc.gpsimd.memset(ones[:], 1.0)
        nc.gpsimd.memset(v_ap()[:, :, :, HS:VW], 1.0)

        # PE warmup: dependency-free matmuls on memset data ramp the PE to
        # full clock while the x/weight DMAs are in flight, so the first
        # real projection runs at 2.4 GHz instead of the cold p-state.
        wu = smp.tile([P, TBLK], BF16, tag="bcast", name="warmup_in")
        nc.gpsimd.memset(wu[:], 0.0)
        wups = psC.tile([P, TBLK], F32, tag="small", name="warmup_ps")
        NWU = 8
        for i in range(NWU):
            nc.tensor.matmul(
                wups[:], lhsT=wu[:, 0:P], rhs=wu[:],
                start=(i == 0), stop=(i == NWU - 1),
            )
        wuo = smp.tile([1, TBLK], F32, tag="recip", name="warmup_out")
        nc.vector.tensor_copy(wuo[:], wups[0:1, :])

        def x_dma(tb):
            nc.sync.dma_start(
                x_ap()[:, :, :, tb],
                x8[:].rearrange("(ci p) (hl tb t) -> p hl ci tb t",
                                p=P, hl=2, tb=NTB)[:, :, :, tb],
            )

        def w_dma(t_sb, d):
            nc.sync.dma_start(
                t_sb[:].rearrange("p (ci f) -> p ci f", ci=NCT),
                d[:].rearrange("(ci p) f -> p ci f", p=P),
            )

        # wq + the first x t-block first, so stage-1 starts ASAP; wv before
        # x1 so the V(tb0) chunks don't stall the early-loop PE
        w_dma(wqh_sb, wqh)
        x_dma(0)
        w_dma(wql_sb, wql)
        nc.sync.dma_start(tri_sb[:], tri2[:])
        w_dma(wkh_sb, wkh)
        w_dma(wkl_sb, wkl)
        x_dma(1)
        w_dma(wvh_sb, wvh)
        w_dma(wvl_sb, wvl)
        x_dma(2)
        x_dma(3)
        nc.sync.dma_start(
            wo_sb[:].rearrange("p (ci f) -> p ci f", ci=NCT),
            wo[:].rearrange("(ci p) f -> p ci f", p=P),
        )
        nc.sync.dma_start(bias_sb[:], bo[:])
        nc.gpsimd.partition_broadcast(bias_bc[:], bias_sb[:])

        for chunk in qk_chunks(0) + v_chunks(0):
            chunk()

        def drive_pair(qb, pr, attn_pair, vfill, fillers, stride, off=0):
            """Drive one head pair's s-loop, weaving V fillers (odd units)
            and other fillers (every `stride` units after `off`)."""
            ctr = 0
            for _ in emit_headpair(qb, pr, attn_pair):
                ctr += 1
                if vfill and ctr % 2 == 1:
                    vfill.pop(0)()
                elif (fillers and ctr > off
                      and (ctr - off) % stride == 0):
                    fillers.pop(0)()

        lh_of = {}
        ap_of = {}

        def new_attn_pair(qb):
            ap_of[qb] = [
                attp.tile([P, TBLK], BF16, tag=f"attn{p_}",
                          name=f"at{qb}_{p_}")
                for p_ in range(2)
            ]
            lh_of[qb] = [None] * NCT
            return ap_of[qb]

        def oproj_fillers(qb):
            return [(lambda tt=tt, q=qb: emit_oproj_tt(q, lh_of[q], tt))
                    for tt in range(4)]

        # ---- t-blocks 0 and 1: sequential. Stage-1 chunks (QK/V of later
        # t-blocks) fill these early loops; all out-projections are deferred
        # to the late loops, which have no stage-1 work left.
        ap0, ap1 = new_attn_pair(0), new_attn_pair(1)
        f0 = qk_chunks(1) + v_chunks(1)
        drive_pair(0, 0, ap0[0], [], f0, 1)
        emit_ag(0, 0, ap0[0], lh_of[0])
        drive_pair(0, 1, ap0[1], [], f0, 1)
        while f0:
            f0.pop(0)()
        emit_ag(0, 1, ap0[1], lh_of[0])

        f1 = qk_chunks(2) + v_chunks(2)
        drive_pair(1, 0, ap1[0], [], f1, 1)
        emit_ag(1, 0, ap1[0], lh_of[1])
        drive_pair(1, 1, ap1[1], [], f1, 1)
        while f1:
            f1.pop(0)()
        emit_ag(1, 1, ap1[1], lh_of[1])

        # ---- t-blocks 2 and 3: interleaved at head-pair granularity.
        # Remaining stage-1 work and the deferred out-projections are
        # spread across these ACT-bound loops to keep the PE fed.
        # ---- late pairs: the next pair's first scores are emitted BEFORE
        # the previous pair's AV-drain + normalize, so the ACT exp pipeline
        # never starves across pair boundaries.
        ap2, ap3 = new_attn_pair(2), new_attn_pair(3)
        PRE = 3

        def pump(gen, n):
            for _ in range(n):
                next(gen)

        def run_yields(gen, n, ctr, fillers, stride, off=0):
            for _ in range(n):
                next(gen)
                ctr += 1
                if fillers and ctr > off and (ctr - off) % stride == 0:
                    fillers.pop(0)()

        def finish(gen, fillers=None):
            for _ in gen:
                pass
            while fillers:
                fillers.pop(0)()

        f23 = qk_chunks(3) + v_chunks(3)
        g20 = emit_headpair(2, 0, ap2[0])
        run_yields(g20, 12, 0, f23, 1)
        g30 = emit_headpair(3, 0, ap3[0])
        pump(g30, PRE)
        finish(g20, f23)
        emit_ag(2, 0, ap2[0], lh_of[2])

        of0 = oproj_fillers(0)
        run_yields(g30, 16 - PRE, PRE, of0, 4)
        g21 = emit_headpair(2, 1, ap2[1])
        pump(g21, PRE)
        finish(g30, of0)
        emit_ag(3, 0, ap3[0], lh_of[3])

        of1 = oproj_fillers(1)
        run_yields(g21, 12 - PRE, PRE, of1, 3)
        g31 = emit_headpair(3, 1, ap3[1])
        pump(g31, PRE)
        finish(g21, of1)
        emit_ag(2, 1, ap2[1], lh_of[2])

        # oproj(2) is NOT woven into the (3,1) loop (which is ACT-bound —
        # PE filler is wasted there); it is held back to cover the final
        # AllGather's flight in the tail, where PE would otherwise idle.
        run_yields(g31, 16 - PRE, PRE, [], 99)
        finish(g31)
        of2 = oproj_fillers(2)

        # tail: out-projection of the last t-block; bias + pr0 hd-tiles
        # first so PE has work while the final AllGather is in flight.
        lhz = lh_of[NTB - 1]
        tz = (NTB - 1) * TBLK

        def open_tail_evens(tts):
            ops = {}
            for tt in tts:
                # tail groups live in the st2 banks: free after the last
                # exp, so their WAR guard resolves at exp time rather than
                # at the end of the final pair's normalize chain
                op = ps2.tile([P, ES], F32, tag="st2", name=f"opz{tt}")
                nc.tensor.matmul(
                    op[:], lhsT=ones[0:1, :], rhs=bias_sb[:],
                    start=True, stop=False,
                )
                for hdt in (0, 2, 4, 6):
                    nc.tensor.matmul(
                        op[:],
                        lhsT=lh_slice(lhz, hdt, tt * P, (tt + 1) * P),
                        rhs=wo_ap()[:, hdt],
                        start=False, stop=False,
                    )
                ops[tt] = op
            return ops

        # the first two t-tiles' even-half groups are emitted BEFORE the
        # final AllGather so their semaphore thresholds (and hence start
        # time) pre-date the gather chain they don't depend on
        ops = open_tail_evens((0, 1))
        for f in of2:
            f()
        emit_ag(3, 1, ap3[1], lh_of[3], last=True)
        for grp in range(2):
            tts = (2 * grp, 2 * grp + 1)
            if grp == 1:
                ops = open_tail_evens(tts)
            for tt in tts:
                for j, hdt in enumerate((1, 3, 5, 7)):
                    nc.tensor.matmul(
                        ops[tt][:],
                        lhsT=lh_slice(lhz, hdt, tt * P, (tt + 1) * P),
                        rhs=wo_ap()[:, hdt],
                        start=False, stop=(j == 3),
                    )
                osb = outp.tile([P, ES], F32, tag="osb", name=f"osbz{tt}")
                # endgame: ACT/DVE alternate so the final stores drain in
                # parallel instead of serializing on one engine
                if tt % 2 == 0:
                    nc.scalar.activation(
                        osb[:], ops[tt][:],
                        mybir.ActivationFunctionType.Copy, scale=1.0)
                    nc.scalar.dma_start(
                        out[tz + tt * P:tz + (tt + 1) * P, :], osb[:])
                else:
                    nc.vector.tensor_copy(osb[:], ops[tt][:])
                    nc.sync.dma_start(
                        out[tz + tt * P:tz + (tt + 1) * P, :], osb[:])

    nc.compile()
    return nc


_NC_CACHE = {}


def _get_nc(with_collective=True):
    key = with_collective
    if key not in _NC_CACHE:
        _NC_CACHE[key] = build_nc(with_collective)
    return _NC_CACHE[key]


def _f8(a):
    return a.astype(E4)


def _split8(a, scale):
    hi = _f8(scale * a)
    lo = _f8(scale * a - hi.astype(np.float32))
    return hi, lo


def make_in_maps(x, Wq, Wk, Wv, Wo, bo):
    tri = np.triu(np.ones((P, P), dtype=np.float32))
    tri2 = np.concatenate([tri, tri], axis=1).astype(BF)
    in_maps = []
    for c in range(N_CORES):
        b, g = c // GROUPS, c % GROUPS
        hs_ = slice(g * HPG, (g + 1) * HPG)

        # x8: [C, hl(2), tb(4), t(512)] -> [C, 2T]
        xT = np.ascontiguousarray(x[b].T)            # [C, T]
        x_hi, x_lo = _split8(xT, SX)
        x8 = np.stack([x_hi, x_lo], axis=1)          # [C, 2, T]
        x8 = x8.reshape(C, 2, NTB, TBLK).reshape(C, 2 * T)

        def prep_w(W):
            # W[hs_] -> [C, HD] in (pr, par, hs) column order
            Wl = W[hs_].transpose(1, 0, 2).reshape(C, HD)
            hi, lo = _split8(Wl, SW)
            # hi duplicated per pr block: [C, pr, 2, 128]
            hid = hi.reshape(C, 2, P)
            hid = np.stack([hid, hid], axis=2).reshape(C, 2 * HD)
            return np.ascontiguousarray(hid), np.ascontiguousarray(lo)

        wqh_, wql_ = prep_w(Wq)
        wkh_, wkl_ = prep_w(Wk)
        # V: hi duplicated as one [C, 2, 256] block (no pr split)
        Wvl_ = Wv[hs_].transpose(1, 0, 2).reshape(C, HD)
        v_hi, v_lo = _split8(Wvl_, SW)
        wvh_ = np.ascontiguousarray(
            np.stack([v_hi, v_hi], axis=1).reshape(C, 2 * HD))

        in_maps.append({
            "x8": np.ascontiguousarray(x8),
            "wqh": wqh_, "wkh": wkh_, "wvh": wvh_,
            "wql": wql_, "wkl": wkl_,
            "wvl": np.ascontiguousarray(v_lo),
            "wo": np.ascontiguousarray(Wo[:, g * ES:(g + 1) * ES]).astype(BF),
            "bo": np.ascontiguousarray(
                bo[g * ES:(g + 1) * ES].reshape(1, ES)).astype(BF),
            "tri2": tri2,
        })
    return in_maps


def kernel(x, Wq, Wk, Wv, Wo, bo):
    x = np.asarray(x, dtype=np.float32)
    Wq = np.asarray(Wq, dtype=np.float32)
    Wk = np.asarray(Wk, dtype=np.float32)
    Wv = np.asarray(Wv, dtype=np.float32)
    Wo = np.asarray(Wo, dtype=np.float32)
    bo = np.asarray(bo, dtype=np.float32)

    nc = _get_nc(with_collective=True)
    in_maps = make_in_maps(x, Wq, Wk, Wv, Wo, bo)
    res = run_bass_kernel_spmd(nc, in_maps, core_ids=list(range(N_CORES)))

    out = np.empty((B, T, E), dtype=np.float32)
    for c in range(N_CORES):
        b, g = c // GROUPS, c % GROUPS
        out[b, :, g * ES:(g + 1) * ES] = res.results[c]["out"]
    return out
